# revision 16
# baseline (speedup 1.0000x reference)
"""BWGNN (Beta Wavelet GNN) Trainium2 kernel, 8-way SPMD.

Math (reference.py): deg = out-degree(src) clamped >=1; Dinv = deg^-1/2;
h = leaky_relu(feature @ W + b); L feat = feat - Dinv*segsum_dst(Dinv[src]*feat[src]);
out = concat_i sum_k THETA[i][k] L^k h.

We iterate on u_k = Dinv * L^k h:
    u_{k+1} = u_k - Dinv^2 * segsum_dst(u_k[src])
    out_i   = (sum_k THETA[i][k] u_k) * deg^{1/2}

Distribution (v3): edges are partitioned by SOURCE core; each core keeps only
its OWN u slab as a local 256B-row bf16 table and gathers per-edge messages
from it (no cross-core u table, no AllGather). Per hop every core computes
partial dst aggregates for ALL 784 global 128-node dst windows (one-hot
matmuls into PSUM, fp16 partials staged to DRAM), and four per-quarter
ReduceScatters sum the partials across cores and hand each core the
aggregates for its own nodes. Each quarter's ReduceScatter launches as soon
as its 4 phases of windows finish, so the collectives pipeline inside the
hop instead of serializing at the hop boundary.

Key performance structure:
 - Per-edge u[src] rows pulled by dma_gather (SWDGE, int16 local idx, 4
   queues) from the core's own 3.2MB table.
 - dst nodes are assigned to 128-node windows by a host-side balancing pass
   so nearly every (window, src-core) bucket fits in 2 tiles of 128 edges.
 - One-hot S matrices built on the fly by the DVE (iota is_equal against a
   pairwise-duplicated dst-loc table -- the [stride 1, count 2] innermost
   dim keeps the op in the DVE 2x perf mode).
 - PE matmuls (bf16 lhsT=S, rhs=gathered msgs) accumulate windows in PSUM;
   ACT casts PSUM->fp16 partials; after each quarter's ReduceScatter, ACT
   scales by Dinv^2 and DVE subtracts into the slab.
 - Features arrive pre-transposed so the u0 stage needs no PE transposes.
"""

import math
import os
import sys

sys.path.insert(0, "/opt/trn_rl_repo")

import numpy as np

# ---------------------------------------------------------------- constants
N = 100000
E = 1600000
F_IN = 128
H = 64
NCORES = 8
NPC = 12500          # nodes per core
WPC = 98             # windows (128-node groups) per core
NPC_PAD = WPC * 128  # 12544
NTAB = NCORES * NPC_PAD  # 100352
NQ = 4               # ReduceScatter quarters per hop
QW = [25, 25, 24, 24]        # my windows per quarter
QW0 = [0, 25, 50, 74]        # first local window of each quarter
PPQ = 4              # phases per quarter
NPHASE = NQ * PPQ    # 16
SBATCH = 16          # S tiles built per DVE instruction
NHOP = 3
G_EPI = 7            # windows per epilogue combine block


def _calculate_theta2(d):
    thetas = []
    for i in range(d):
        c1 = np.zeros(i + 1)
        c1[i] = 0.5 ** i
        c2 = np.array([math.comb(d - i, j) * (-0.5) ** j for j in range(d - i + 1)])
        c = np.convolve(c1, c2)
        B = math.factorial(i) * math.factorial(d - i) / math.factorial(d + 1)
        c = c / (2.0 * B)
        thetas.append([float(c[j]) for j in range(d)])
    return thetas


THETAS = _calculate_theta2(4)  # [4][4], theta[i][k] weight of L^k h in output i


# ---------------------------------------------------------------- host prep
def _balance_windows(indeg, nwin, over_idx, cap, over_cap):
    """Assign each dst core's nodes to `nwin` windows keeping per-(window,
    src-core) counts under `cap` except overflow windows (`over_cap`).
    Greedy + swap repair. indeg: [NCORES, NPC, ND] per-node indegree split
    over ND dims. Returns assign [NCORES, NPC] -> window."""
    ND = indeg.shape[2]
    CAP = np.full((nwin, ND), cap, np.int64)
    CAP[list(over_idx), :] = over_cap
    assign = np.full((NCORES, NPC), -1, np.int32)
    for c in range(NCORES):
        nodes_c = np.arange(NPC)
        d_vec = indeg[c].astype(np.int64)
        order = nodes_c[np.argsort(-d_vec.sum(1), kind="stable")]
        L = np.zeros((nwin, ND), np.int64)
        counts = np.zeros(nwin, np.int64)
        for n in order:
            dv = d_vec[n]
            excess = np.maximum(L + dv - CAP, 0).sum(axis=1)
            tot = (L + dv).max(axis=1)
            score = excess * 100000 + tot
            score[counts >= 128] = 1 << 60
            ww = int(np.argmin(score))
            L[ww] += dv
            counts[ww] += 1
            assign[c, n] = ww
        for _sweep in range(6):
            L = np.zeros((nwin, ND), np.int64)
            for kk in range(ND):
                np.add.at(L[:, kk], assign[c], d_vec[:, kk])
            over = np.argwhere(L > CAP)
            if len(over) == 0:
                break
            win_nodes = [np.where(assign[c] == ww)[0] for ww in range(nwin)]
            for ww, kk in over:
                while L[ww, kk] > CAP[ww, kk]:
                    nodes = win_nodes[ww]
                    nodes = nodes[d_vec[nodes, kk] > 0]
                    if len(nodes) == 0:
                        break
                    n = nodes[np.argmax(d_vec[nodes, kk])]
                    dn = d_vec[n]
                    done = False
                    for w2 in np.argsort(L[:, kk]):
                        if w2 == ww:
                            continue
                        cand = win_nodes[w2]
                        if len(cand) == 0:
                            continue
                        dm = d_vec[cand]
                        ok = ((L[w2] + dn - dm) <= CAP[w2]).all(1) & \
                             (dm[:, kk] < dn[kk])
                        ok &= ((L[ww] - dn + dm) <= CAP[ww]).all(1) | (
                            ((L[ww] - dn + dm) < L[ww]).any(1)
                            & (dm[:, kk] < dn[kk])
                        )
                        if ok.any():
                            m = cand[np.argmax(ok)]
                            assign[c, n], assign[c, m] = w2, ww
                            L[ww] += d_vec[m] - dn
                            L[w2] += dn - d_vec[m]
                            win_nodes[ww] = np.where(assign[c] == ww)[0]
                            win_nodes[w2] = np.where(assign[c] == w2)[0]
                            done = True
                            break
                    if not done:
                        break
    return assign


def _prep(edge_index: np.ndarray):
    """Bucket src-partitioned edges by global dst window, build per-core
    gather-index / dst-loc arrays and the shared tile-count table."""
    src = edge_index[0].astype(np.int64)
    dst = edge_index[1].astype(np.int64)

    deg = np.bincount(src, minlength=N).astype(np.float32)
    dinv = np.maximum(deg, np.float32(1.0)) ** np.float32(-0.5)
    dinv2 = dinv * dinv
    dsqrt = np.float32(1.0) / dinv  # = max(deg,1)^0.5

    dcore = dst // NPC
    n_loc = dst % NPC
    owner = src // NPC  # edge owner = src core

    # balance dst windows against per-src-core indegree (8 dims, cap 256)
    indeg8 = np.zeros((NCORES, NPC, NCORES), np.int32)
    np.add.at(indeg8, (dcore, n_loc, owner), 1)
    assign = _balance_windows(
        indeg8, WPC, (0, 28, 56, 84), 256, 384
    ).astype(np.int64)

    # pos[c][n] = window*128 + slot
    pos = np.zeros((NCORES, NPC), np.int64)
    for c in range(NCORES):
        order = np.argsort(assign[c], kind="stable")
        slot = np.zeros(NPC, np.int64)
        ww = assign[c][order]
        first = np.searchsorted(ww, np.arange(WPC), side="left")
        slot[order] = np.arange(NPC) - first[ww]
        pos[c] = assign[c] * 128 + slot

    # global window order: quarter-major [q][dst core][local w]
    # gwi(c, w) = processing index of window (c, w)
    gwi_map = np.zeros((NCORES, WPC), np.int64)
    win_list = []  # gwi -> (q, c, w)
    for q in range(NQ):
        for c in range(NCORES):
            for w in range(QW0[q], QW0[q] + QW[q]):
                gwi_map[c, w] = len(win_list)
                win_list.append((q, c, w))
    NWIN = len(win_list)
    assert NWIN == NCORES * WPC

    w_of_dst = assign[dcore, n_loc]
    gwi_e = gwi_map[dcore, w_of_dst]
    dst_loc = pos[dcore, n_loc] % 128
    src_pos = pos[owner, src % NPC]  # local row in the owner's table

    bucket = owner * NWIN + gwi_e
    cnt = np.bincount(bucket, minlength=NCORES * NWIN).reshape(NCORES, NWIN)
    T = np.maximum(1, -(-cnt // 128)).max(axis=0)  # [NWIN]

    order = np.argsort(bucket, kind="stable")
    src_pos_s = src_pos[order]
    dst_loc_s = dst_loc[order]
    starts = np.zeros(NCORES * NWIN + 1, dtype=np.int64)
    np.cumsum(np.bincount(bucket[order], minlength=NCORES * NWIN),
              out=starts[1:])

    tot_tiles = int(T.sum())
    tot_slots = tot_tiles * 128
    tile_start = np.zeros(NWIN + 1, dtype=np.int64)
    np.cumsum(T, out=tile_start[1:])

    # phases: per quarter, split that quarter's windows into PPQ equal runs
    phases = []  # [p] = (gwi0, n_win, tile0, n_tiles)
    for q in range(NQ):
        qwin = NCORES * QW[q]
        g0 = sum(NCORES * QW[qq] for qq in range(q))
        step = qwin // PPQ
        assert step * PPQ == qwin
        for j in range(PPQ):
            a = g0 + j * step
            b = a + step
            phases.append(
                (a, step, int(tile_start[a]), int(tile_start[b] - tile_start[a]))
            )
    max_ptiles = max(p[3] for p in phases)

    # per-core slot arrays
    import ml_dtypes

    per_core = []
    for c in range(NCORES):
        idx_arr = np.zeros(tot_slots, dtype=np.int16)
        dl_arr = np.full(tot_slots, -1, dtype=np.int64)
        for g in range(NWIN):
            b = c * NWIN + g
            s0, s1 = starts[b], starts[b + 1]
            n = s1 - s0
            o = int(tile_start[g]) * 128
            cap = int(T[g]) * 128
            assert n <= cap
            idx_arr[o : o + n] = src_pos_s[s0:s1]
            dl_arr[o : o + n] = dst_loc_s[s0:s1]
        idx_w = idx_arr.reshape(-1, 16).T
        idx_w = np.tile(idx_w, (8, 1))  # [128, tot/16]
        dl_t = np.repeat(
            dl_arr.reshape(tot_tiles, 128).T.astype(ml_dtypes.bfloat16), 2, axis=1
        )
        per_core.append((idx_w, np.ascontiguousarray(dl_t)))

    def slice_arr(a):
        out = np.ones((NCORES, NPC_PAD), dtype=np.float32)
        av = a.reshape(NCORES, NPC)
        for c in range(NCORES):
            out[c, pos[c]] = av[c]
        return out.reshape(NCORES, WPC, 128).transpose(0, 2, 1).copy()

    return {
        "pos": pos,
        "T": T,
        "tot_tiles": tot_tiles,
        "phases": phases,
        "max_ptiles": max_ptiles,
        "tile_start": tile_start,
        "win_list": win_list,
        "per_core": per_core,
        "dinv_t": slice_arr(dinv),
        "dinv2_t": slice_arr(dinv2),
        "dsqrt_t": slice_arr(dsqrt),
    }


# ---------------------------------------------------------------- bass build
def _build_nc(prep, reps=1):
    import concourse.bacc as bacc
    import concourse.mybir as mybir
    import concourse.tile as tile
    from concourse.library_config import mlp

    f32 = mybir.dt.float32
    bf16 = mybir.dt.bfloat16
    i16 = mybir.dt.int16

    tot_tiles = prep["tot_tiles"]
    tot16 = tot_tiles * 128 // 16

    nc = bacc.Bacc("TRN2", target_bir_lowering=False, debug=False,
                   num_devices=NCORES, num_swdge_queues=4)

    featT_in = nc.dram_tensor("featT_in", [F_IN, NPC_PAD], f32, kind="ExternalInput")
    w_in = nc.dram_tensor("w_in", [F_IN, H], f32, kind="ExternalInput")
    b_in = nc.dram_tensor("b_in", [1, H], f32, kind="ExternalInput")
    iota_in = nc.dram_tensor("iota_in", [128, SBATCH * 128], bf16,
                             kind="ExternalInput")
    idx_in = nc.dram_tensor("idx_in", [128, tot16], i16, kind="ExternalInput")
    dl_in = nc.dram_tensor("dl_in", [128, 2 * tot_tiles], bf16,
                           kind="ExternalInput")
    dinv_in = nc.dram_tensor("dinv_in", [128, WPC], f32, kind="ExternalInput")
    dinv2_in = nc.dram_tensor("dinv2_in", [128, WPC], f32, kind="ExternalInput")
    dsqrt_in = nc.dram_tensor("dsqrt_in", [128, WPC], f32, kind="ExternalInput")
    out = nc.dram_tensor("out", [NPC_PAD, 4 * H], f32, kind="ExternalOutput")

    with tile.TileContext(nc) as tc:
        with (
            tc.tile_pool(name="dram", bufs=1, space="DRAM") as dram,
            tc.tile_pool(name="const", bufs=1) as const,
            tc.tile_pool(name="slabs", bufs=1) as slabs,
            tc.tile_pool(name="work", bufs=3) as work,
            tc.tile_pool(name="msgs_p", bufs=2) as msgs_pool,
            tc.tile_pool(name="psum", bufs=4, space="PSUM") as psum_pool,
        ):
            nc.gpsimd.load_library(mlp)

            w_sb = const.tile([F_IN, H], f32)
            nc.sync.dma_start(out=w_sb[:], in_=w_in[:])
            b_sb = const.tile([1, H], f32)
            nc.sync.dma_start(out=b_sb[:], in_=b_in[:])
            iota_sb = const.tile([128, SBATCH * 128], bf16)
            nc.sync.dma_start(out=iota_sb[:], in_=iota_in[:])
            dinv_sb = const.tile([128, WPC], f32)
            nc.sync.dma_start(out=dinv_sb[:], in_=dinv_in[:])
            dinv2_sb = const.tile([128, WPC], f32)
            nc.sync.dma_start(out=dinv2_sb[:], in_=dinv2_in[:])
            dsqrt_sb = const.tile([128, WPC], f32)
            nc.sync.dma_start(out=dsqrt_sb[:], in_=dsqrt_in[:])
            ones_col = const.tile([1, 128], f32)
            nc.vector.memset(ones_col[:], 1.0)

            slab_a = slabs.tile([128, WPC * H], f32)  # u0 -> u2
            slab_b = slabs.tile([128, WPC * H], f32)  # u1 -> u3

            saves = [
                dram.tile([128, WPC * H], f32, name=f"save{kk}") for kk in range(2)
            ]

            for rep in range(reps):
                _emit_body(
                    nc, tc, mybir, rep, prep, dram, work, msgs_pool, psum_pool,
                    slab_a, slab_b, saves, featT_in, out, w_sb, b_sb,
                    iota_sb, idx_in, dl_in, dinv_sb, dinv2_sb, dsqrt_sb,
                    ones_col,
                )

    nc.compile()
    return nc


def _emit_body(
    nc, tc, mybir, rep, prep, dram, work, msgs_pool, psum_pool, slab_a,
    slab_b, saves, featT_in, out, w_sb, b_sb, iota_sb, idx_in, dl_in,
    dinv_sb, dinv2_sb, dsqrt_sb, ones_col,
):
    import concourse.mybir as mybir

    f32 = mybir.dt.float32
    bf16 = mybir.dt.bfloat16
    fp16 = mybir.dt.float16
    i16 = mybir.dt.int16
    Alu = mybir.AluOpType

    T = prep["T"]
    phases = prep["phases"]
    max_ptiles = prep["max_ptiles"]
    tile_start = prep["tile_start"]
    win_list = prep["win_list"]
    max_pwin = max(p[1] for p in phases)

    # two alternating local u tables (256B rows; only first 64 cols written)
    tables = [
        dram.tile([NPC_PAD, 2 * H], bf16, name=f"tbl{rep}_{i}") for i in range(2)
    ]
    # per-quarter partial-aggregate buffers and ReduceScatter outputs
    sz_q = [NCORES * QW[q] * 128 for q in range(NQ)]
    partials = [
        dram.tile([sz_q[q], H], fp16, name=f"part{rep}_{q}") for q in range(NQ)
    ]
    aggs = [
        dram.tile([QW[q] * 128, H], fp16, name=f"agg{rep}_{q}")
        for q in range(NQ)
    ]

    def write_table(slab, tbl, q):
        """cast my slab's quarter-q windows to bf16 and store into the local
        table rows (strided 256B rows, 128B payload)."""
        w0, nw = QW0[q], QW[q]
        stq = work.tile([128, max(QW) * H], bf16, tag="stq", bufs=2)
        nc.vector.tensor_copy(
            out=stq[:, : nw * H], in_=slab[:, w0 * H : (w0 + nw) * H]
        )
        nc.sync.dma_start(
            out=tbl[w0 * 128 : (w0 + nw) * 128, :H].rearrange(
                "(w p) h -> p w h", p=128
            ),
            in_=stq[:, : nw * H].rearrange("p (w h) -> p w h", h=H),
        )

    if True:  # keep indentation style of prior version
            # ---------- u0 slab (own nodes only) + local table 0
            with tc.tile_pool(name=f"featp{rep}", bufs=2) as featp:
                for w0 in range(0, WPC, 7):
                    gw = min(7, WPC - w0)
                    fTl = featp.tile([128, 7 * 128], f32, tag="fTl")
                    nc.sync.dma_start(
                        out=fTl[:, : gw * 128],
                        in_=featT_in[:, w0 * 128 : (w0 + gw) * 128],
                    )
                    for w in range(w0, w0 + gw):
                        h_ps = psum_pool.tile([128, 128], f32, tag="ps", bufs=8)
                        nc.tensor.matmul(
                            out=h_ps[:, :H],
                            lhsT=fTl[:, (w - w0) * 128 : (w - w0 + 1) * 128],
                            rhs=w_sb[:], start=True, stop=False,
                        )
                        nc.tensor.matmul(
                            out=h_ps[:, :H], lhsT=ones_col[:], rhs=b_sb[:],
                            start=False, stop=True,
                        )
                        t2 = work.tile([128, H], f32, tag="t2")
                        nc.scalar.activation(
                            out=t2[:], in_=h_ps[:, :H],
                            func=mybir.ActivationFunctionType.Lrelu, alpha=0.01,
                        )
                        nc.vector.tensor_scalar(
                            out=slab_a[:, w * H : (w + 1) * H], in0=t2[:],
                            scalar1=dinv_sb[:, w : w + 1], scalar2=None,
                            op0=Alu.mult,
                        )
            for q in range(NQ):
                write_table(slab_a, tables[0], q)
            nc.sync.dma_start(out=saves[0][:], in_=slab_a[:])

            # ---------- hops
            cur, nxt = slab_a, slab_b
            for hop in range(NHOP):
                rd = tables[hop % 2]
                wr = tables[(hop + 1) % 2]
                for p in range(NPHASE):
                    gwi0, n_win, tile0, n_tiles = phases[p]
                    q = p // PPQ
                    p_off16 = tile0 * 128 // 16
                    p_len16 = n_tiles * 128 // 16
                    idxp = msgs_pool.tile(
                        [128, (max_ptiles * 128) // 16], i16, tag="idxp"
                    )
                    nc.sync.dma_start(
                        out=idxp[:, :p_len16],
                        in_=idx_in[:, p_off16 : p_off16 + p_len16],
                    )
                    dlp = msgs_pool.tile([128, 2 * max_ptiles], bf16, tag="dlp")
                    nc.sync.dma_start(
                        out=dlp[:, : 2 * n_tiles],
                        in_=dl_in[:, 2 * tile0 : 2 * (tile0 + n_tiles)],
                    )
                    s_built = msgs_pool.tile(
                        [128, max_ptiles * 128], bf16, tag="sb", bufs=1
                    )
                    for t0 in range(0, n_tiles, SBATCH):
                        tb = min(SBATCH, n_tiles - t0)
                        nc.vector.tensor_tensor(
                            out=s_built[:, t0 * 128 : (t0 + tb) * 128].rearrange(
                                "p (t jh jl) -> p t jh jl", jl=2, jh=64
                            ),
                            in0=iota_sb[:, : tb * 128].rearrange(
                                "p (t jh jl) -> p t jh jl", jl=2, jh=64
                            ),
                            in1=dlp[:, 2 * t0 : 2 * (t0 + tb)].rearrange(
                                "p (t o jl) -> p t o jl", jl=2, o=1
                            ).to_broadcast([128, tb, 64, 2]),
                            op=Alu.is_equal,
                        )
                    msgs16 = msgs_pool.tile(
                        [128, max_ptiles, 2 * H], bf16, tag="msgs16"
                    )
                    # 4 gather calls (one per SWDGE queue) over the phase's
                    # tile range
                    tsplit = [n_tiles * k // 4 for k in range(5)]
                    for kk in range(4):
                        ta, tb_ = tsplit[kk], tsplit[kk + 1]
                        if tb_ == ta:
                            continue
                        ln = (tb_ - ta) * 128
                        nc.gpsimd.dma_gather(
                            msgs16[:, ta:tb_, :],
                            rd[:, :],
                            idxp[:, ta * 8 : ta * 8 + ln // 16],
                            ln,
                            ln,
                            2 * H,
                            single_packet=False,
                            queue_num=kk,
                        )
                    # matmuls + PSUM->fp16 partial stage per window
                    pstage = msgs_pool.tile(
                        [128, max_pwin, H], fp16, tag="pst"
                    )
                    for j in range(n_win):
                        g = gwi0 + j
                        agg_ps = psum_pool.tile([128, 128], f32, tag="ps",
                                                bufs=8)
                        n_mm = int(T[g])
                        t_base = int(tile_start[g]) - tile0
                        for t in range(n_mm):
                            nc.tensor.matmul(
                                out=agg_ps[:, :H],
                                lhsT=s_built[
                                    :, (t_base + t) * 128 : (t_base + t + 1) * 128
                                ],
                                rhs=msgs16[:, t_base + t, :H],
                                start=(t == 0),
                                stop=(t == n_mm - 1),
                            )
                        nc.scalar.activation(
                            out=pstage[:, j, :], in_=agg_ps[:, :H],
                            func=mybir.ActivationFunctionType.Copy,
                        )
                    # one partial-store per phase (contiguous rows)
                    row0 = (gwi0 - sum(NCORES * QW[qq] for qq in range(q))) * 128
                    nc.sync.dma_start(
                        out=partials[q][row0 : row0 + n_win * 128, :].rearrange(
                            "(w p) h -> p w h", p=128
                        ),
                        in_=pstage[:, :n_win, :],
                    )
                    if p % PPQ == PPQ - 1:
                        # quarter complete on every core -> ReduceScatter
                        nc.gpsimd.collective_compute(
                            "ReduceScatter",
                            Alu.add,
                            replica_groups=[list(range(NCORES))],
                            ins=[partials[q].opt()],
                            outs=[aggs[q].opt()],
                        )
                        # update my windows of this quarter
                        w0, nw = QW0[q], QW[q]
                        agg_sb = work.tile([128, max(QW) * H], fp16,
                                           tag="aggsb", bufs=2)
                        nc.sync.dma_start(
                            out=agg_sb[:, : nw * H].rearrange(
                                "p (w h) -> p w h", h=H
                            ),
                            in_=aggs[q][:, :].rearrange(
                                "(w p) h -> p w h", p=128
                            ),
                        )
                        for w in range(w0, w0 + nw):
                            tscl = work.tile([128, H], f32, tag="tscl")
                            nc.scalar.activation(
                                out=tscl[:],
                                in_=agg_sb[:, (w - w0) * H : (w - w0 + 1) * H],
                                func=mybir.ActivationFunctionType.Copy,
                                scale=dinv2_sb[:, w : w + 1],
                            )
                            nc.vector.tensor_tensor(
                                out=nxt[:, w * H : (w + 1) * H],
                                in0=cur[:, w * H : (w + 1) * H],
                                in1=tscl[:],
                                op=Alu.subtract,
                            )
                        if hop < NHOP - 1:
                            write_table(nxt, wr, q)
                        if hop == NHOP - 1:
                            # epilogue combine for this quarter's windows
                            for e0 in range(w0, w0 + nw, G_EPI):
                                ge = min(G_EPI, w0 + nw - e0)
                                cs = slice(e0 * H, (e0 + ge) * H)
                                u0c = work.tile([128, G_EPI * H], f32,
                                                tag="u0c", bufs=2)
                                nc.sync.dma_start(out=u0c[:, : ge * H],
                                                  in_=saves[0][:, cs])
                                u1c = work.tile([128, G_EPI * H], f32,
                                                tag="u1c", bufs=2)
                                nc.sync.dma_start(out=u1c[:, : ge * H],
                                                  in_=saves[1][:, cs])
                                us = [u0c[:, : ge * H], u1c[:, : ge * H],
                                      cur[:, cs], nxt[:, cs]]
                                out_st = work.tile([128, G_EPI, 4 * H], f32,
                                                   tag="outst", bufs=2)
                                for i in range(4):
                                    acc = work.tile([128, G_EPI * H], f32,
                                                    tag="acc", bufs=2)
                                    a = acc[:, : ge * H]
                                    ks = [kk for kk in range(4)
                                          if THETAS[i][kk] != 0.0]
                                    nc.scalar.activation(
                                        out=a, in_=us[ks[0]],
                                        func=mybir.ActivationFunctionType.Copy,
                                        scale=float(THETAS[i][ks[0]]),
                                    )
                                    for kk in ks[1:]:
                                        tmp = work.tile([128, G_EPI * H], f32,
                                                        tag="ctmp", bufs=2)
                                        nc.scalar.activation(
                                            out=tmp[:, : ge * H], in_=us[kk],
                                            func=mybir.ActivationFunctionType.Copy,
                                            scale=float(THETAS[i][kk]),
                                        )
                                        nc.vector.tensor_tensor(
                                            out=a, in0=a, in1=tmp[:, : ge * H],
                                            op=Alu.add,
                                        )
                                    nc.vector.tensor_tensor(
                                        out=out_st[:, :ge, i * H : (i + 1) * H],
                                        in0=a.rearrange("p (w h) -> p w h", h=H),
                                        in1=dsqrt_sb[:, e0 : e0 + ge]
                                        .to_broadcast([128, ge, H]),
                                        op=Alu.mult,
                                    )
                                nc.sync.dma_start(
                                    out=out[
                                        e0 * 128 : (e0 + ge) * 128, :
                                    ].rearrange("(w p) h -> p w h", p=128),
                                    in_=out_st[:, :ge, :],
                                )
                if hop == 0:
                    nc.sync.dma_start(out=saves[1][:], in_=nxt[:])  # save u1
                cur, nxt = nxt, cur


# ---------------------------------------------------------------- runner
def _make_runner(nc, in_maps, n_cores):
    import jax
    import numpy as np
    from jax.sharding import Mesh, NamedSharding, PartitionSpec
    from jax.experimental.shard_map import shard_map

    import concourse.mybir as mybir
    from concourse import bass2jax

    bass2jax.install_neuronx_cc_hook()
    partition_name = nc.partition_id_tensor.name if nc.partition_id_tensor else None
    in_names, out_names, out_avals, zero_outs = [], [], [], []
    for alloc in nc.m.functions[0].allocations:
        if not isinstance(alloc, mybir.MemoryLocationSet):
            continue
        name = alloc.memorylocations[0].name
        if alloc.kind == "ExternalInput":
            if name != partition_name:
                in_names.append(name)
        elif alloc.kind == "ExternalOutput":
            out_names.append(name)
            shape = tuple(alloc.tensor_shape)
            dtype = mybir.dt.np(alloc.dtype)
            out_avals.append(jax.core.ShapedArray(shape, dtype))
            zero_outs.append(np.zeros(shape, dtype))
    n_params = len(in_names)
    all_in_names = list(in_names) + list(out_names)
    if partition_name is not None:
        all_in_names.append(partition_name)

    def _body(*args):
        operands = list(args)
        if partition_name is not None:
            operands.append(bass2jax.partition_id_tensor())
        outs = bass2jax._bass_exec_p.bind(
            *operands,
            out_avals=tuple(out_avals),
            in_names=tuple(all_in_names),
            out_names=tuple(out_names),
            lowering_input_output_aliases=(),
            sim_require_finite=True,
            sim_require_nnan=True,
            nc=nc,
        )
        return tuple(outs)

    devices = jax.devices()[:n_cores]
    mesh = Mesh(np.asarray(devices), ("core",))
    n_ops = n_params + len(out_names)
    sharded = jax.jit(
        shard_map(
            _body,
            mesh=mesh,
            in_specs=(PartitionSpec("core"),) * n_ops,
            out_specs=(PartitionSpec("core"),) * len(out_names),
            check_rep=False,
        ),
        keep_unused=True,
    )
    sh = NamedSharding(mesh, PartitionSpec("core"))
    concat_in = [
        jax.device_put(
            np.concatenate([np.asarray(in_maps[c][nm]) for c in range(n_cores)], 0),
            sh,
        )
        for nm in in_names
    ]
    concat_zeros = [
        jax.device_put(np.zeros((n_cores * z.shape[0], *z.shape[1:]), z.dtype), sh)
        for z in zero_outs
    ]
    args = concat_in + concat_zeros

    def run():
        return sharded(*args)

    return run, out_names, out_avals


_CACHE = {}


def _get_built(edge_index_bytes_key, edge_index):
    if edge_index_bytes_key not in _CACHE:
        prep = _prep(edge_index)
        nc = _build_nc(prep)
        _CACHE[edge_index_bytes_key] = (prep, nc)
    return _CACHE[edge_index_bytes_key]


def _make_in_maps(prep, inputs):
    import ml_dtypes

    feature = np.asarray(inputs["feature"], dtype=np.float32)
    W = np.asarray(inputs["W"], dtype=np.float32)
    b = np.asarray(inputs["b"], dtype=np.float32)

    b2 = b.reshape(1, H)
    iota = np.tile(np.arange(128, dtype=np.float32), SBATCH).reshape(1, -1)
    iota = np.repeat(iota, 128, axis=0).astype(ml_dtypes.bfloat16)

    pos = prep["pos"]
    feat_pad = np.zeros((NCORES, NPC_PAD, F_IN), dtype=np.float32)
    fv = feature.reshape(NCORES, NPC, F_IN)
    for c in range(NCORES):
        feat_pad[c, pos[c], :] = fv[c]
    featT = np.ascontiguousarray(feat_pad.transpose(0, 2, 1))

    in_maps = []
    for c in range(NCORES):
        idx_w, dl_t = prep["per_core"][c]
        in_maps.append(
            {
                "featT_in": featT[c],
                "w_in": W,
                "b_in": b2,
                "iota_in": iota,
                "idx_in": idx_w,
                "dl_in": dl_t,
                "dinv_in": prep["dinv_t"][c],
                "dinv2_in": prep["dinv2_t"][c],
                "dsqrt_in": prep["dsqrt_t"][c],
            }
        )
    return in_maps


def kernel(feature, edge_index, W, b):
    import jax

    edge_index = np.asarray(edge_index, dtype=np.int32)
    key = hash(edge_index.tobytes())
    prep, nc = _get_built(key, edge_index)
    in_maps = _make_in_maps(prep, {"feature": feature, "W": W, "b": b})

    run, out_names, out_avals = _make_runner(nc, in_maps, NCORES)
    outs = jax.block_until_ready(run())
    oi = out_names.index("out")
    full = np.asarray(outs[oi]).reshape(NCORES, NPC_PAD, 4 * H)
    pos = prep["pos"]
    res = np.empty((NCORES, NPC, 4 * H), np.float32)
    for c in range(NCORES):
        res[c] = full[c, pos[c], :]
    return res.reshape(N, 4 * H)


if __name__ == "__main__":
    rng = np.random.default_rng(0)
    feature = rng.standard_normal((N, F_IN), dtype=np.float32)
    edge_index = rng.integers(0, N, (2, E)).astype(np.int32)
    W = (rng.standard_normal((F_IN, H)) * 0.05).astype(np.float32)
    b = (rng.standard_normal((H,)) * 0.05).astype(np.float32)
    out = kernel(feature=feature, edge_index=edge_index, W=W, b=b)
    print(out.shape, out.dtype, float(np.abs(out).mean()))


# revision 17
# speedup vs baseline: 1.1836x; 1.1836x over previous
"""BWGNN (Beta Wavelet GNN) Trainium2 kernel, 8-way SPMD.

Math (reference.py): deg = out-degree(src) clamped >=1; Dinv = deg^-1/2;
h = leaky_relu(feature @ W + b); L feat = feat - Dinv*segsum_dst(Dinv[src]*feat[src]);
out = concat_i sum_k THETA[i][k] L^k h.

We iterate on u_k = Dinv * L^k h:
    u_{k+1} = u_k - Dinv^2 * segsum_dst(u_k[src])
    out_i   = (sum_k THETA[i][k] u_k) * deg^{1/2}

Distribution: nodes dst-sharded over 8 cores (12500 + pad -> 12544 rows/core).
Full u-table (bf16, 256B rows = 64 values + 64 pad cols) lives in each core's
HBM; tables 1,2 are refreshed per hop by an intra-chip AllGather of the
per-core bf16 slices.  Table 0 (u0) is built LOCALLY on every core from a
replicated transposed bf16 copy of the full feature matrix -- no AllGather
for hop 1.

Key performance structure:
 - Per-edge u[src] rows are pulled by dma_gather (SWDGE, int16 local idx)
   spread across all 4 SWDGE queues (queue_num=chunk) so descriptor
   generation runs on all 4 Q7 core-pairs concurrently.
 - Nodes are assigned to 128-node dst-windows by a host-side balancing pass
   (greedy + swap repair) so nearly every (window, src-chunk) bucket fits in
   4 tiles of 128 edges (~2% slot padding).
 - The segment-sum one-hot S matrices are built ON THE FLY by the DVE
   (iota is_equal against a per-slot dst-loc table, 8 tiles per op) --
   replaces streaming 52MB/hop of host-built one-hots from HBM.
 - PE matmuls (bf16 lhsT=S, rhs=gathered msgs) accumulate each window in
   PSUM; the PSUM scale by Dinv^2 runs on the Activation engine; DVE does
   the S build, subtract, u0 activation scale and the final combine.
 - Features arrive pre-transposed ([F, nodes]) so the u0 stage needs no PE
   transposes; outputs of the 4 wavelets are staged and stored with one
   1KB-per-row DMA per phase.
"""

import math
import os
import sys

sys.path.insert(0, "/opt/trn_rl_repo")

import numpy as np

# ---------------------------------------------------------------- constants
N = 100000
E = 1600000
F_IN = 128
H = 64
NCORES = 8
NPC = 12500          # nodes per core
WPC = 98             # windows (128-node groups) per core
NPC_PAD = WPC * 128  # 12544
NCHUNK = 4
CH_NODES = 25000     # original nodes per chunk
CH_PAD = 2 * NPC_PAD  # 25088 padded rows per chunk
NTAB = NCORES * NPC_PAD  # 100352
GW_ALL = NTAB // 128     # 784 global windows
G_WIN = 7            # windows per phase
NPHASE = WPC // G_WIN  # 14
SBATCH = 16          # S tiles built per DVE instruction
NHOP = 3


def _calculate_theta2(d):
    thetas = []
    for i in range(d):
        c1 = np.zeros(i + 1)
        c1[i] = 0.5 ** i
        c2 = np.array([math.comb(d - i, j) * (-0.5) ** j for j in range(d - i + 1)])
        c = np.convolve(c1, c2)
        B = math.factorial(i) * math.factorial(d - i) / math.factorial(d + 1)
        c = c / (2.0 * B)
        thetas.append([float(c[j]) for j in range(d)])
    return thetas


THETAS = _calculate_theta2(4)  # [4][4], theta[i][k] weight of L^k h in output i


# ---------------------------------------------------------------- host prep
def _balance_windows(indeg, half, nwin, over_idx):
    """Assign nodes to `nwin` windows keeping per-(window,chunk) counts under
    512 except the overflow windows in `over_idx` (640). Greedy + swap repair.
    indeg: [NCORES, NPC, NCHUNK]; half: [NCORES, NPC] bool mask of nodes to
    place. Returns assign [NCORES, NPC] -> window in [0, nwin)."""
    WPC_, NCHUNK_ = nwin, NCHUNK
    CAP = np.full((WPC_, NCHUNK_), 512, np.int64)
    CAP[list(over_idx), :] = 640
    assign = np.full((NCORES, NPC), -1, np.int32)
    for c in range(NCORES):
        nodes_c = np.where(half[c])[0]
        d_all = indeg[c].astype(np.int64)
        d_vec = d_all
        order = nodes_c[np.argsort(-d_all[nodes_c].sum(1), kind="stable")]
        L = np.zeros((WPC_, NCHUNK_), np.int64)
        counts = np.zeros(WPC_, np.int64)
        for n in order:
            dv = d_vec[n]
            excess = np.maximum(L + dv - CAP, 0).sum(axis=1)
            tot = (L + dv).max(axis=1)
            score = excess * 100000 + tot
            score[counts >= 128] = 1 << 60
            ww = int(np.argmin(score))
            L[ww] += dv
            counts[ww] += 1
            assign[c, n] = ww
        for _sweep in range(6):
            L = np.zeros((WPC_, NCHUNK_), np.int64)
            for kk in range(NCHUNK):
                np.add.at(L[:, kk], assign[c][nodes_c], d_vec[nodes_c, kk])
            over = np.argwhere(L > CAP)
            if len(over) == 0:
                break
            win_nodes = [np.where(assign[c] == ww)[0] for ww in range(WPC_)]
            for ww, kk in over:
                while L[ww, kk] > CAP[ww, kk]:
                    nodes = win_nodes[ww]
                    nodes = nodes[d_vec[nodes, kk] > 0]
                    if len(nodes) == 0:
                        break
                    n = nodes[np.argmax(d_vec[nodes, kk])]
                    dn = d_vec[n]
                    done = False
                    for w2 in np.argsort(L[:, kk]):
                        if w2 == ww:
                            continue
                        cand = win_nodes[w2]
                        if len(cand) == 0:
                            continue
                        dm = d_vec[cand]
                        ok = ((L[w2] + dn - dm) <= CAP[w2]).all(1) & \
                             (dm[:, kk] < dn[kk])
                        ok &= ((L[ww] - dn + dm) <= CAP[ww]).all(1) | (
                            ((L[ww] - dn + dm) < L[ww]).any(1)
                            & (dm[:, kk] < dn[kk])
                        )
                        if ok.any():
                            m = cand[np.argmax(ok)]
                            assign[c, n], assign[c, m] = w2, ww
                            L[ww] += d_vec[m] - dn
                            L[w2] += dn - d_vec[m]
                            win_nodes[ww] = np.where(assign[c] == ww)[0]
                            win_nodes[w2] = np.where(assign[c] == w2)[0]
                            done = True
                            break
                    if not done:
                        break
    return assign


def _prep(edge_index: np.ndarray):
    """Bucket edges, build per-core gather-index / dst-loc arrays and the
    static tile-count table T[w][k] (shared by all cores)."""
    src = edge_index[0].astype(np.int64)
    dst = edge_index[1].astype(np.int64)

    deg = np.bincount(src, minlength=N).astype(np.float32)
    dinv = np.maximum(deg, np.float32(1.0)) ** np.float32(-0.5)
    dinv2 = dinv * dinv
    dsqrt = np.float32(1.0) / dinv  # = max(deg,1)^0.5

    core = dst // NPC
    n_loc = dst % NPC
    src_core = src // NPC
    k = src // CH_NODES

    indeg = np.zeros((NCORES, NPC, NCHUNK), np.int32)
    np.add.at(indeg, (core, n_loc, k), 1)
    # spread the 4 overflow windows across phases so no phase is inflated
    assign = _balance_windows(
        indeg, np.ones((NCORES, NPC), bool), WPC, (0, 28, 56, 84)
    ).astype(np.int64)

    # pos[c][n] = window*128 + slot (slot = rank within window, <128)
    pos = np.zeros((NCORES, NPC), np.int64)
    for c in range(NCORES):
        order = np.argsort(assign[c], kind="stable")
        slot = np.zeros(NPC, np.int64)
        ww = assign[c][order]
        first = np.searchsorted(ww, np.arange(WPC), side="left")
        slot[order] = np.arange(NPC) - first[ww]
        pos[c] = assign[c] * 128 + slot

    w = assign[core, n_loc]
    dst_loc = pos[core, n_loc] % 128
    src_pos = pos[src_core, src % NPC]
    src_loc = (src_core % 2) * NPC_PAD + src_pos

    bucket = ((core * WPC + w) * NCHUNK + k)
    cnt = np.bincount(bucket, minlength=NCORES * WPC * NCHUNK).reshape(
        NCORES, WPC, NCHUNK
    )
    T = np.maximum(1, -(-cnt // 128)).max(axis=0)  # [WPC, NCHUNK] int64

    # sort edges by bucket (stable, any order within bucket)
    order = np.argsort(bucket, kind="stable")
    src_loc_s = src_loc[order]
    dst_loc_s = dst_loc[order]
    bucket_s = bucket[order]
    starts = np.zeros(NCORES * WPC * NCHUNK + 1, dtype=np.int64)
    np.cumsum(np.bincount(bucket_s, minlength=NCORES * WPC * NCHUNK), out=starts[1:])

    tot_tiles = int(T.sum())
    tot_slots = tot_tiles * 128

    # per-(p,k): column base within phase msgs tile and call length
    phase_tiles = []  # [p] -> total tiles in phase
    call_info = []  # [p][k] = (idx_off_slots, n_slots, col_base)
    gcol = 0
    for p in range(NPHASE):
        ws = range(p * G_WIN, (p + 1) * G_WIN)
        info = []
        col = 0
        for kk in range(NCHUNK):
            n_t = int(sum(T[ww][kk] for ww in ws))
            info.append((gcol * 128, n_t * 128, col))
            col += n_t
            gcol += n_t
        call_info.append(info)
        phase_tiles.append(col)
    assert gcol == tot_tiles

    # map (w,k) -> global tile start
    tile_start = np.zeros((WPC, NCHUNK), dtype=np.int64)
    g = 0
    for p in range(NPHASE):
        for kk in range(NCHUNK):
            for ww in range(p * G_WIN, (p + 1) * G_WIN):
                tile_start[ww][kk] = g
                g += T[ww][kk]

    # build per-core slot arrays
    import ml_dtypes

    per_core = []
    for c in range(NCORES):
        idx_arr = np.zeros(tot_slots, dtype=np.int16)
        dl_arr = np.full(tot_slots, -1, dtype=np.int64)
        for ww in range(WPC):
            for kk in range(NCHUNK):
                b = (c * WPC + ww) * NCHUNK + kk
                s0, s1 = starts[b], starts[b + 1]
                n = s1 - s0
                o = tile_start[ww][kk] * 128
                cap = T[ww][kk] * 128
                assert n <= cap
                idx_arr[o : o + n] = src_loc_s[s0:s1]
                dl_arr[o : o + n] = dst_loc_s[s0:s1]
                if n < cap:  # pad with a valid idx (row 0 of chunk), dst -1
                    idx_arr[o + n : o + cap] = 0
        # wrap idx: position i -> [16r + i%16, i//16]
        idx_w = idx_arr.reshape(-1, 16).T  # [16, tot/16]
        idx_w = np.tile(idx_w, (8, 1))  # [128, tot/16]
        # per-slot dst-loc table for on-device S build: dl_t[p, 2g] =
        # dl_t[p, 2g+1] = dst loc of slot g*128+p (or -1 for pad slots, which
        # never match iota). Pairwise duplication gives the DVE is_equal a
        # stride-1 innermost dim (2x perf mode) instead of a pure broadcast.
        dl_t = np.repeat(
            dl_arr.reshape(tot_tiles, 128).T.astype(ml_dtypes.bfloat16), 2, axis=1
        )
        per_core.append((idx_w, np.ascontiguousarray(dl_t)))

    # per-core dinv arrays [128, WPC] (pad nodes -> 1.0), permuted to pos order
    def slice_arr(a):
        out = np.ones((NCORES, NPC_PAD), dtype=np.float32)
        av = a.reshape(NCORES, NPC)
        for c in range(NCORES):
            out[c, pos[c]] = av[c]
        return out.reshape(NCORES, WPC, 128).transpose(0, 2, 1).copy()

    dinv_t = slice_arr(dinv)
    # dinv in global (core, pos) order for the full u0 sweep: [128, GW_ALL]
    dinv_full = np.concatenate([dinv_t[c] for c in range(NCORES)], axis=1)

    return {
        "pos": pos,
        "T": T,
        "tot_tiles": tot_tiles,
        "phase_tiles": phase_tiles,
        "call_info": call_info,
        "tile_start": tile_start,
        "per_core": per_core,
        "dinv_t": dinv_t,
        "dinv2_t": slice_arr(dinv2),
        "dsqrt_t": slice_arr(dsqrt),
        "dinv_full": np.ascontiguousarray(dinv_full),
    }


# ---------------------------------------------------------------- bass build
def _build_nc(T, tot_tiles, phase_tiles, call_info, tile_start, reps=1):
    import concourse.bacc as bacc
    import concourse.mybir as mybir
    import concourse.tile as tile
    from concourse.library_config import mlp

    f32 = mybir.dt.float32
    bf16 = mybir.dt.bfloat16
    i16 = mybir.dt.int16

    tot16 = tot_tiles * 128 // 16

    nc = bacc.Bacc("TRN2", target_bir_lowering=False, debug=False,
                   num_devices=NCORES, num_swdge_queues=4)

    featT_in = nc.dram_tensor("featT_in", [F_IN, NPC_PAD], f32, kind="ExternalInput")
    featfullT_in = nc.dram_tensor(
        "featfullT_in", [F_IN, NTAB], bf16, kind="ExternalInput"
    )
    w_in = nc.dram_tensor("w_in", [F_IN, H], f32, kind="ExternalInput")
    b_in = nc.dram_tensor("b_in", [1, H], f32, kind="ExternalInput")
    iota_in = nc.dram_tensor("iota_in", [128, SBATCH * 128], bf16,
                             kind="ExternalInput")
    idx_in = nc.dram_tensor("idx_in", [128, tot16], i16, kind="ExternalInput")
    dl_in = nc.dram_tensor("dl_in", [128, 2 * tot_tiles], bf16,
                           kind="ExternalInput")
    dinv_in = nc.dram_tensor("dinv_in", [128, WPC], f32, kind="ExternalInput")
    dinv2_in = nc.dram_tensor("dinv2_in", [128, WPC], f32, kind="ExternalInput")
    dsqrt_in = nc.dram_tensor("dsqrt_in", [128, WPC], f32, kind="ExternalInput")
    dinvfull_in = nc.dram_tensor("dinvfull_in", [128, GW_ALL], f32,
                                 kind="ExternalInput")
    out = nc.dram_tensor("out", [NPC_PAD, 4 * H], f32, kind="ExternalOutput")

    SLAB = WPC * H  # 6272 free elems

    with tile.TileContext(nc) as tc:
        with (
            tc.tile_pool(name="dram", bufs=1, space="DRAM") as dram,
            tc.tile_pool(name="const", bufs=1) as const,
            tc.tile_pool(name="slabs", bufs=1) as slabs,
            tc.tile_pool(name="work", bufs=3) as work,
            tc.tile_pool(name="msgs_p", bufs=2) as msgs_pool,
            tc.tile_pool(name="psum", bufs=4, space="PSUM") as psum_pool,
        ):
            nc.gpsimd.load_library(mlp)

            # ---------- constants / metadata loads
            w_sb = const.tile([F_IN, H], f32)
            nc.sync.dma_start(out=w_sb[:], in_=w_in[:])
            w16 = const.tile([F_IN, H], bf16)
            nc.vector.tensor_copy(out=w16[:], in_=w_sb[:])
            b_sb = const.tile([1, H], f32)
            nc.sync.dma_start(out=b_sb[:], in_=b_in[:])
            b16 = const.tile([1, H], bf16)
            nc.vector.tensor_copy(out=b16[:], in_=b_sb[:])
            iota_sb = const.tile([128, SBATCH * 128], bf16)
            nc.sync.dma_start(out=iota_sb[:], in_=iota_in[:])
            dinv_sb = const.tile([128, WPC], f32)
            nc.sync.dma_start(out=dinv_sb[:], in_=dinv_in[:])
            dinv2_sb = const.tile([128, WPC], f32)
            nc.sync.dma_start(out=dinv2_sb[:], in_=dinv2_in[:])
            dsqrt_sb = const.tile([128, WPC], f32)
            nc.sync.dma_start(out=dsqrt_sb[:], in_=dsqrt_in[:])
            dinvfull_sb = const.tile([128, GW_ALL], f32)
            nc.sync.dma_start(out=dinvfull_sb[:], in_=dinvfull_in[:])
            ones_col = const.tile([1, 128], f32)
            nc.vector.memset(ones_col[:], 1.0)
            ones16 = const.tile([1, 128], bf16)
            nc.vector.memset(ones16[:], 1.0)

            slab_a = slabs.tile([128, SLAB], f32)  # u0 -> u2
            slab_b = slabs.tile([128, SLAB], f32)  # u1 -> u3

            saves = [
                dram.tile([128, SLAB], f32, name=f"save{kk}") for kk in range(2)
            ]
            ag_h = dram.tile([NPC_PAD, 2 * H], mybir.dt.bfloat16, name="agin")
            stage16 = slabs.tile([128, SLAB], mybir.dt.bfloat16, name="stg")

            for rep in range(reps):
                _emit_body(
                    nc, tc, mybir, rep, T, tot_tiles, phase_tiles, call_info,
                    tile_start, dram, work, msgs_pool, psum_pool, slab_a, slab_b,
                    saves, ag_h, featT_in, featfullT_in, out, w_sb, b_sb,
                    w16, b16, iota_sb, idx_in, dl_in, dinv_sb, dinv2_sb,
                    dsqrt_sb, dinvfull_sb, ones_col, ones16, stage16,
                )

    nc.compile()
    return nc


def _emit_body(
    nc, tc, mybir, rep, T, tot_tiles, phase_tiles, call_info, tile_start,
    dram, work, msgs_pool, psum_pool, slab_a, slab_b, saves, ag_h, featT_in,
    featfullT_in, out, w_sb, b_sb, w16, b16, iota_sb, idx_in, dl_in,
    dinv_sb, dinv2_sb, dsqrt_sb, dinvfull_sb, ones_col, ones16, stage16,
):
    import concourse.mybir as mybir

    f32 = mybir.dt.float32
    bf16 = mybir.dt.bfloat16
    i16 = mybir.dt.int16
    Alu = mybir.AluOpType
    SLAB = WPC * H
    max_ptiles = max(phase_tiles)

    tables = [
        dram.tile(
            [NTAB, 2 * H], mybir.dt.bfloat16,
            **({} if hop == 0 else {"addr_space": "Shared"}),
            name=f"tbl{rep}_{hop}",
        )
        for hop in range(NHOP)
    ]

    def launch_ag(hop_dst):
        nc.gpsimd.collective_compute(
            "AllGather",
            mybir.AluOpType.bypass,
            replica_groups=[list(range(NCORES))],
            ins=[ag_h.opt()],
            outs=[tables[hop_dst].opt()],
        )

    if True:  # keep indentation of original body
            # ---------- u0 table (ALL nodes, bf16) built locally; plus the
            # local f32 u0 slab for this core's own windows.
            GB = 8  # global windows per feature-stream batch
            with tc.tile_pool(name=f"featp{rep}", bufs=2) as featp:
                # full sweep in global (chunk-major) order -> tables[0]
                for g0 in range(0, GW_ALL, GB):
                    gb = min(GB, GW_ALL - g0)
                    fT = featp.tile([128, GB * 128], bf16, tag="fT")
                    nc.sync.dma_start(
                        out=fT[:, : gb * 128],
                        in_=featfullT_in[:, g0 * 128 : (g0 + gb) * 128],
                    )
                    u0st = featp.tile([128, GB, H], bf16, tag="u0st")
                    for j in range(gb):
                        gw = g0 + j
                        h_ps = psum_pool.tile([128, 128], f32, tag="ps", bufs=8)
                        nc.tensor.matmul(
                            out=h_ps[:, :H], lhsT=fT[:, j * 128 : (j + 1) * 128],
                            rhs=w16[:], start=True, stop=False,
                        )
                        nc.tensor.matmul(
                            out=h_ps[:, :H], lhsT=ones16[:], rhs=b16[:],
                            start=False, stop=True,
                        )
                        t2 = work.tile([128, H], f32, tag="t2")
                        nc.scalar.activation(
                            out=t2[:], in_=h_ps[:, :H],
                            func=mybir.ActivationFunctionType.Lrelu, alpha=0.01,
                        )
                        nc.vector.tensor_scalar(
                            out=u0st[:, j, :], in0=t2[:],
                            scalar1=dinvfull_sb[:, gw : gw + 1], scalar2=None,
                            op0=Alu.mult,
                        )
                    nc.sync.dma_start(
                        out=tables[0][g0 * 128 : (g0 + gb) * 128, :H].rearrange(
                            "(w p) h -> p w h", p=128
                        ),
                        in_=u0st[:, :gb, :],
                    )
                # local f32 slab for this core's own nodes
                for w0 in range(0, WPC, G_WIN):
                    gw = min(G_WIN, WPC - w0)
                    fTl = featp.tile([128, G_WIN * 128], f32, tag="fTl")
                    nc.sync.dma_start(
                        out=fTl[:, : gw * 128],
                        in_=featT_in[:, w0 * 128 : (w0 + gw) * 128],
                    )
                    for w in range(w0, w0 + gw):
                        h_ps = psum_pool.tile([128, 128], f32, tag="ps", bufs=8)
                        nc.tensor.matmul(
                            out=h_ps[:, :H],
                            lhsT=fTl[:, (w - w0) * 128 : (w - w0 + 1) * 128],
                            rhs=w_sb[:], start=True, stop=False,
                        )
                        nc.tensor.matmul(
                            out=h_ps[:, :H], lhsT=ones_col[:], rhs=b_sb[:],
                            start=False, stop=True,
                        )
                        t2 = work.tile([128, H], f32, tag="t2")
                        nc.scalar.activation(
                            out=t2[:], in_=h_ps[:, :H],
                            func=mybir.ActivationFunctionType.Lrelu, alpha=0.01,
                        )
                        nc.vector.tensor_scalar(
                            out=slab_a[:, w * H : (w + 1) * H], in0=t2[:],
                            scalar1=dinv_sb[:, w : w + 1], scalar2=None,
                            op0=Alu.mult,
                        )
            nc.sync.dma_start(out=saves[0][:], in_=slab_a[:])

            # ---------- hops
            cur, nxt = slab_a, slab_b
            for hop in range(NHOP):
                g = 0  # global tile counter
                for p in range(NPHASE):
                    ptiles = phase_tiles[p]
                    p_off16 = call_info[p][0][0] // 16  # phase idx col start
                    p_len16 = ptiles * 128 // 16
                    first_g_p = call_info[p][0][0] // 128
                    idxp = msgs_pool.tile(
                        [128, (max_ptiles * 128) // 16], i16, tag="idxp"
                    )
                    nc.sync.dma_start(
                        out=idxp[:, :p_len16],
                        in_=idx_in[:, p_off16 : p_off16 + p_len16],
                    )
                    # on-the-fly S build: load per-slot dst-locs (pairwise
                    # duplicated), compare against iota (SBATCH tiles per DVE
                    # op). The [stride 1, count 2] innermost dim on in1 keeps
                    # the op eligible for the DVE 2x perf mode.
                    dlp = msgs_pool.tile([128, 2 * max_ptiles], bf16, tag="dlp")
                    nc.sync.dma_start(
                        out=dlp[:, : 2 * ptiles],
                        in_=dl_in[:, 2 * first_g_p : 2 * (first_g_p + ptiles)],
                    )
                    s_built = msgs_pool.tile(
                        [128, max_ptiles * 128], bf16, tag="sb", bufs=1
                    )
                    for t0 in range(0, ptiles, SBATCH):
                        tb = min(SBATCH, ptiles - t0)
                        nc.vector.tensor_tensor(
                            out=s_built[:, t0 * 128 : (t0 + tb) * 128].rearrange(
                                "p (t jh jl) -> p t jh jl", jl=2, jh=64
                            ),
                            in0=iota_sb[:, : tb * 128].rearrange(
                                "p (t jh jl) -> p t jh jl", jl=2, jh=64
                            ),
                            in1=dlp[:, 2 * t0 : 2 * (t0 + tb)].rearrange(
                                "p (t o jl) -> p t o jl", jl=2, o=1
                            ).to_broadcast([128, tb, 64, 2]),
                            op=Alu.is_equal,
                        )
                    msgs16 = msgs_pool.tile(
                        [128, max_ptiles, 2 * H], bf16, tag="msgs16"
                    )
                    GCAP = 8192  # max idxs per dma_gather (desc-ring capacity)
                    for kk in range(NCHUNK):
                        off_sl, n_sl, col = call_info[p][kk]
                        for o in range(0, n_sl, GCAP):
                            ln = min(GCAP, n_sl - o)
                            c0 = col + o // 128
                            i0 = (off_sl + o) // 16 - p_off16
                            nc.gpsimd.dma_gather(
                                msgs16[:, c0 : c0 + ln // 128, :],
                                tables[hop][CH_PAD * kk : CH_PAD * (kk + 1), :],
                                idxp[:, i0 : i0 + ln // 16],
                                ln,
                                ln,
                                2 * H,
                                single_packet=False,
                                queue_num=kk,
                            )
                    first_g = g
                    # matmuls per window
                    for ww in range(p * G_WIN, (p + 1) * G_WIN):
                        agg_ps = psum_pool.tile([128, 128], f32, tag="ps", bufs=8)
                        n_mm = int(sum(T[ww][kk] for kk in range(NCHUNK)))
                        mm_i = 0
                        for kk in range(NCHUNK):
                            _, _, col = call_info[p][kk]
                            cbase = col + int(
                                sum(T[w2][kk] for w2 in range(p * G_WIN, ww))
                            )
                            for t in range(int(T[ww][kk])):
                                # global tile index in host (p,k,w,t) order
                                lg = int(tile_start[ww][kk]) + t - first_g
                                nc.tensor.matmul(
                                    out=agg_ps[:, :H],
                                    lhsT=s_built[:, lg * 128 : (lg + 1) * 128],
                                    rhs=msgs16[:, cbase + t, :H],
                                    start=(mm_i == 0),
                                    stop=(mm_i == n_mm - 1),
                                )
                                mm_i += 1
                        g += n_mm
                        # u' = u - dinv2 * agg  (scale on ACT, subtract on DVE)
                        tscl = work.tile([128, H], f32, tag="tscl")
                        nc.scalar.activation(
                            out=tscl[:], in_=agg_ps[:, :H],
                            func=mybir.ActivationFunctionType.Copy,
                            scale=dinv2_sb[:, ww : ww + 1],
                        )
                        nc.vector.tensor_tensor(
                            out=nxt[:, ww * H : (ww + 1) * H],
                            in0=cur[:, ww * H : (ww + 1) * H],
                            in1=tscl[:],
                            op=Alu.subtract,
                        )
                    if hop < NHOP - 1:
                        # pre-cast this phase's u slice to bf16 and stage it
                        # into ag_h right away, so the hop boundary pays
                        # nothing before the AllGather launches
                        cs2 = slice(p * G_WIN * H, (p + 1) * G_WIN * H)
                        nc.vector.tensor_copy(
                            out=stage16[:, cs2], in_=nxt[:, cs2]
                        )
                        w0p = p * G_WIN
                        nc.sync.dma_start(
                            out=ag_h[
                                w0p * 128 : (w0p + G_WIN) * 128, :H
                            ].rearrange("(w p) h -> p w h", p=128),
                            in_=stage16[:, cs2].rearrange(
                                "p (w h) -> p w h", h=H
                            ),
                        )
                    if hop == NHOP - 1:
                        # combine this phase's windows now (u3 slice just
                        # written; u0/u1 from DRAM, u2 = cur) -- hides the
                        # epilogue under the gather-bound last hop
                        w0 = p * G_WIN
                        cs = slice(w0 * H, (w0 + G_WIN) * H)
                        u0c = work.tile([128, G_WIN * H], f32, tag="u0c",
                                        bufs=2)
                        nc.sync.dma_start(out=u0c[:], in_=saves[0][:, cs])
                        u1c = work.tile([128, G_WIN * H], f32, tag="u1c",
                                        bufs=2)
                        nc.sync.dma_start(out=u1c[:], in_=saves[1][:, cs])
                        us = [u0c[:], u1c[:], cur[:, cs], nxt[:, cs]]
                        out_st = work.tile([128, G_WIN, 4 * H], f32,
                                           tag="outst", bufs=2)
                        for i in range(4):
                            acc = work.tile([128, G_WIN * H], f32, tag="acc",
                                            bufs=2)
                            a = acc[:]
                            ks = [kk for kk in range(4)
                                  if THETAS[i][kk] != 0.0]
                            nc.scalar.activation(
                                out=a, in_=us[ks[0]],
                                func=mybir.ActivationFunctionType.Copy,
                                scale=float(THETAS[i][ks[0]]),
                            )
                            for kk in ks[1:]:
                                tmp = work.tile([128, G_WIN * H], f32,
                                                tag="ctmp", bufs=2)
                                nc.scalar.activation(
                                    out=tmp[:], in_=us[kk],
                                    func=mybir.ActivationFunctionType.Copy,
                                    scale=float(THETAS[i][kk]),
                                )
                                nc.vector.tensor_tensor(
                                    out=a, in0=a, in1=tmp[:], op=Alu.add
                                )
                            nc.vector.tensor_tensor(
                                out=out_st[:, :, i * H : (i + 1) * H],
                                in0=a.rearrange("p (w h) -> p w h", h=H),
                                in1=dsqrt_sb[:, w0 : w0 + G_WIN].to_broadcast(
                                    [128, G_WIN, H]
                                ),
                                op=Alu.mult,
                            )
                        nc.sync.dma_start(
                            out=out[
                                w0 * 128 : (w0 + G_WIN) * 128, :
                            ].rearrange("(w p) h -> p w h", p=128),
                            in_=out_st[:],
                        )
                assert g == sum(phase_tiles[:NPHASE])
                if hop < NHOP - 1:
                    launch_ag(hop + 1)
                if hop == 0:
                    nc.sync.dma_start(out=saves[1][:], in_=nxt[:])  # save u1
                cur, nxt = nxt, cur


# ---------------------------------------------------------------- runner
def _make_runner(nc, in_maps, n_cores):
    import jax
    import numpy as np
    from jax.sharding import Mesh, NamedSharding, PartitionSpec
    from jax.experimental.shard_map import shard_map

    import concourse.mybir as mybir
    from concourse import bass2jax

    bass2jax.install_neuronx_cc_hook()
    partition_name = nc.partition_id_tensor.name if nc.partition_id_tensor else None
    in_names, out_names, out_avals, zero_outs = [], [], [], []
    for alloc in nc.m.functions[0].allocations:
        if not isinstance(alloc, mybir.MemoryLocationSet):
            continue
        name = alloc.memorylocations[0].name
        if alloc.kind == "ExternalInput":
            if name != partition_name:
                in_names.append(name)
        elif alloc.kind == "ExternalOutput":
            out_names.append(name)
            shape = tuple(alloc.tensor_shape)
            dtype = mybir.dt.np(alloc.dtype)
            out_avals.append(jax.core.ShapedArray(shape, dtype))
            zero_outs.append(np.zeros(shape, dtype))
    n_params = len(in_names)
    all_in_names = list(in_names) + list(out_names)
    if partition_name is not None:
        all_in_names.append(partition_name)

    def _body(*args):
        operands = list(args)
        if partition_name is not None:
            operands.append(bass2jax.partition_id_tensor())
        outs = bass2jax._bass_exec_p.bind(
            *operands,
            out_avals=tuple(out_avals),
            in_names=tuple(all_in_names),
            out_names=tuple(out_names),
            lowering_input_output_aliases=(),
            sim_require_finite=True,
            sim_require_nnan=True,
            nc=nc,
        )
        return tuple(outs)

    devices = jax.devices()[:n_cores]
    mesh = Mesh(np.asarray(devices), ("core",))
    n_ops = n_params + len(out_names)
    sharded = jax.jit(
        shard_map(
            _body,
            mesh=mesh,
            in_specs=(PartitionSpec("core"),) * n_ops,
            out_specs=(PartitionSpec("core"),) * len(out_names),
            check_rep=False,
        ),
        keep_unused=True,
    )
    sh = NamedSharding(mesh, PartitionSpec("core"))
    concat_in = [
        jax.device_put(
            np.concatenate([np.asarray(in_maps[c][nm]) for c in range(n_cores)], 0),
            sh,
        )
        for nm in in_names
    ]
    concat_zeros = [
        jax.device_put(np.zeros((n_cores * z.shape[0], *z.shape[1:]), z.dtype), sh)
        for z in zero_outs
    ]
    args = concat_in + concat_zeros

    def run():
        return sharded(*args)

    return run, out_names, out_avals


_CACHE = {}


def _get_built(edge_index_bytes_key, edge_index):
    if edge_index_bytes_key not in _CACHE:
        prep = _prep(edge_index)
        nc = _build_nc(
            prep["T"],
            prep["tot_tiles"],
            prep["phase_tiles"],
            prep["call_info"],
            prep["tile_start"],
        )
        _CACHE[edge_index_bytes_key] = (prep, nc)
    return _CACHE[edge_index_bytes_key]


def _make_in_maps(prep, inputs):
    import ml_dtypes

    feature = np.asarray(inputs["feature"], dtype=np.float32)
    W = np.asarray(inputs["W"], dtype=np.float32)
    b = np.asarray(inputs["b"], dtype=np.float32)

    b2 = b.reshape(1, H)
    iota = np.tile(np.arange(128, dtype=np.float32), SBATCH).reshape(1, -1)
    iota = np.repeat(iota, 128, axis=0).astype(ml_dtypes.bfloat16)

    pos = prep["pos"]
    feat_pad = np.zeros((NCORES, NPC_PAD, F_IN), dtype=np.float32)
    fv = feature.reshape(NCORES, NPC, F_IN)
    for c in range(NCORES):
        feat_pad[c, pos[c], :] = fv[c]
    # transposed per-core features [F, NPC_PAD] f32
    featT = np.ascontiguousarray(feat_pad.transpose(0, 2, 1))
    # transposed full features (global core-major order), bf16, same for all
    featfullT = np.ascontiguousarray(
        feat_pad.reshape(NCORES * NPC_PAD, F_IN).T.astype(ml_dtypes.bfloat16)
    )

    in_maps = []
    for c in range(NCORES):
        idx_w, dl_t = prep["per_core"][c]
        in_maps.append(
            {
                "featT_in": featT[c],
                "featfullT_in": featfullT,
                "w_in": W,
                "b_in": b2,
                "iota_in": iota,
                "idx_in": idx_w,
                "dl_in": dl_t,
                "dinv_in": prep["dinv_t"][c],
                "dinv2_in": prep["dinv2_t"][c],
                "dsqrt_in": prep["dsqrt_t"][c],
                "dinvfull_in": prep["dinv_full"],
            }
        )
    return in_maps


def kernel(feature, edge_index, W, b):
    import jax

    edge_index = np.asarray(edge_index, dtype=np.int32)
    key = hash(edge_index.tobytes())
    prep, nc = _get_built(key, edge_index)
    in_maps = _make_in_maps(prep, {"feature": feature, "W": W, "b": b})

    run, out_names, out_avals = _make_runner(nc, in_maps, NCORES)
    outs = jax.block_until_ready(run())
    oi = out_names.index("out")
    full = np.asarray(outs[oi]).reshape(NCORES, NPC_PAD, 4 * H)
    pos = prep["pos"]
    res = np.empty((NCORES, NPC, 4 * H), np.float32)
    for c in range(NCORES):
        res[c] = full[c, pos[c], :]
    return res.reshape(N, 4 * H)


if __name__ == "__main__":
    rng = np.random.default_rng(0)
    feature = rng.standard_normal((N, F_IN), dtype=np.float32)
    edge_index = rng.integers(0, N, (2, E)).astype(np.int32)
    W = (rng.standard_normal((F_IN, H)) * 0.05).astype(np.float32)
    b = (rng.standard_normal((H,)) * 0.05).astype(np.float32)
    out = kernel(feature=feature, edge_index=edge_index, W=W, b=b)
    print(out.shape, out.dtype, float(np.abs(out).mean()))


# revision 20
# speedup vs baseline: 1.3228x; 1.1176x over previous
"""BWGNN (Beta Wavelet GNN) Trainium2 kernel, 8-way SPMD.

Math (reference.py): deg = out-degree(src) clamped >=1; Dinv = deg^-1/2;
h = leaky_relu(feature @ W + b); L feat = feat - Dinv*segsum_dst(Dinv[src]*feat[src]);
out = concat_i sum_k THETA[i][k] L^k h.

We iterate on u_k = Dinv * L^k h:
    u_{k+1} = u_k - Dinv^2 * segsum_dst(u_k[src])
    out_i   = (sum_k THETA[i][k] u_k) * deg^{1/2}

Distribution: nodes dst-sharded over 8 cores (12500 + pad -> 12544 rows/core).
Full u-table (bf16, 256B rows = 64 values + 64 pad cols) lives in each core's
HBM; tables 1,2 are refreshed per hop by an intra-chip AllGather of the
per-core bf16 slices.  Table 0 (u0) is built LOCALLY on every core from a
replicated transposed bf16 copy of the full feature matrix -- no AllGather
for hop 1.

Key performance structure:
 - Per-edge u[src] rows are pulled by dma_gather (SWDGE, int16 local idx)
   spread across all 4 SWDGE queues (queue_num=chunk) so descriptor
   generation runs on all 4 Q7 core-pairs concurrently.
 - Nodes are assigned to 128-node dst-windows by a host-side balancing pass
   (greedy + swap repair) so nearly every (window, src-chunk) bucket fits in
   4 tiles of 128 edges (~2% slot padding).
 - The segment-sum one-hot S matrices are built ON THE FLY by the DVE
   (iota is_equal against a per-slot dst-loc table, 8 tiles per op) --
   replaces streaming 52MB/hop of host-built one-hots from HBM.
 - PE matmuls (bf16 lhsT=S, rhs=gathered msgs) accumulate each window in
   PSUM; the PSUM scale by Dinv^2 runs on the Activation engine; DVE does
   the S build, subtract, u0 activation scale and the final combine.
 - Features arrive pre-transposed ([F, nodes]) so the u0 stage needs no PE
   transposes; outputs of the 4 wavelets are staged and stored with one
   1KB-per-row DMA per phase.
"""

import math
import os
import sys

sys.path.insert(0, "/opt/trn_rl_repo")

import numpy as np

# ---------------------------------------------------------------- constants
N = 100000
E = 1600000
F_IN = 128
H = 64
NCORES = 8
NPC = 12500          # nodes per core
WPC = 98             # windows (128-node groups) per core
NPC_PAD = WPC * 128  # 12544
NCHUNK = 4
CH_NODES = 25000     # original nodes per chunk
CH_PAD = 2 * NPC_PAD  # 25088 padded rows per chunk
NTAB = NCORES * NPC_PAD  # 100352
GW_ALL = NTAB // 128     # 784 global windows
G_WIN = 7            # windows per phase
NPHASE = WPC // G_WIN  # 14
SBATCH = 16          # S tiles built per DVE instruction
NHOP = 3


def _calculate_theta2(d):
    thetas = []
    for i in range(d):
        c1 = np.zeros(i + 1)
        c1[i] = 0.5 ** i
        c2 = np.array([math.comb(d - i, j) * (-0.5) ** j for j in range(d - i + 1)])
        c = np.convolve(c1, c2)
        B = math.factorial(i) * math.factorial(d - i) / math.factorial(d + 1)
        c = c / (2.0 * B)
        thetas.append([float(c[j]) for j in range(d)])
    return thetas


THETAS = _calculate_theta2(4)  # [4][4], theta[i][k] weight of L^k h in output i


# ---------------------------------------------------------------- host prep
def _balance_windows(indeg, half, nwin, over_idx):
    """Assign nodes to `nwin` windows keeping per-(window,chunk) counts under
    512 except the overflow windows in `over_idx` (640). Greedy + swap repair.
    indeg: [NCORES, NPC, NCHUNK]; half: [NCORES, NPC] bool mask of nodes to
    place. Returns assign [NCORES, NPC] -> window in [0, nwin)."""
    WPC_, NCHUNK_ = nwin, NCHUNK
    CAP = np.full((WPC_, NCHUNK_), 512, np.int64)
    CAP[list(over_idx), :] = 640
    assign = np.full((NCORES, NPC), -1, np.int32)
    for c in range(NCORES):
        nodes_c = np.where(half[c])[0]
        d_all = indeg[c].astype(np.int64)
        d_vec = d_all
        order = nodes_c[np.argsort(-d_all[nodes_c].sum(1), kind="stable")]
        L = np.zeros((WPC_, NCHUNK_), np.int64)
        counts = np.zeros(WPC_, np.int64)
        for n in order:
            dv = d_vec[n]
            excess = np.maximum(L + dv - CAP, 0).sum(axis=1)
            tot = (L + dv).max(axis=1)
            score = excess * 100000 + tot
            score[counts >= 128] = 1 << 60
            ww = int(np.argmin(score))
            L[ww] += dv
            counts[ww] += 1
            assign[c, n] = ww
        for _sweep in range(20):
            L = np.zeros((WPC_, NCHUNK_), np.int64)
            for kk in range(NCHUNK):
                np.add.at(L[:, kk], assign[c][nodes_c], d_vec[nodes_c, kk])
            over = np.argwhere(L > CAP)
            if len(over) == 0:
                break
            win_nodes = [np.where(assign[c] == ww)[0] for ww in range(WPC_)]
            for ww, kk in over:
                while L[ww, kk] > CAP[ww, kk]:
                    nodes = win_nodes[ww]
                    nodes = nodes[d_vec[nodes, kk] > 0]
                    if len(nodes) == 0:
                        break
                    n = nodes[np.argmax(d_vec[nodes, kk])]
                    dn = d_vec[n]
                    done = False
                    for w2 in np.argsort(L[:, kk]):
                        if w2 == ww:
                            continue
                        cand = win_nodes[w2]
                        if len(cand) == 0:
                            continue
                        dm = d_vec[cand]
                        ok = ((L[w2] + dn - dm) <= CAP[w2]).all(1) & \
                             (dm[:, kk] < dn[kk])
                        ok &= ((L[ww] - dn + dm) <= CAP[ww]).all(1) | (
                            ((L[ww] - dn + dm) < L[ww]).any(1)
                            & (dm[:, kk] < dn[kk])
                        )
                        if ok.any():
                            m = cand[np.argmax(ok)]
                            assign[c, n], assign[c, m] = w2, ww
                            L[ww] += d_vec[m] - dn
                            L[w2] += dn - d_vec[m]
                            win_nodes[ww] = np.where(assign[c] == ww)[0]
                            win_nodes[w2] = np.where(assign[c] == w2)[0]
                            done = True
                            break
                    if not done:
                        break
    return assign


def _prep(edge_index: np.ndarray):
    """Bucket edges, build per-core gather-index / dst-loc arrays and the
    static tile-count table T[w][k] (shared by all cores)."""
    src = edge_index[0].astype(np.int64)
    dst = edge_index[1].astype(np.int64)

    deg = np.bincount(src, minlength=N).astype(np.float32)
    dinv = np.maximum(deg, np.float32(1.0)) ** np.float32(-0.5)
    dinv2 = dinv * dinv
    dsqrt = np.float32(1.0) / dinv  # = max(deg,1)^0.5

    core = dst // NPC
    n_loc = dst % NPC
    src_core = src // NPC
    k = src // CH_NODES

    indeg = np.zeros((NCORES, NPC, NCHUNK), np.int32)
    np.add.at(indeg, (core, n_loc, k), 1)
    # spread the 4 overflow windows across phases so no phase is inflated
    assign = _balance_windows(
        indeg, np.ones((NCORES, NPC), bool), WPC, (0, 28, 56, 84)
    ).astype(np.int64)

    # pos[c][n] = window*128 + slot (slot = rank within window, <128)
    pos = np.zeros((NCORES, NPC), np.int64)
    for c in range(NCORES):
        order = np.argsort(assign[c], kind="stable")
        slot = np.zeros(NPC, np.int64)
        ww = assign[c][order]
        first = np.searchsorted(ww, np.arange(WPC), side="left")
        slot[order] = np.arange(NPC) - first[ww]
        pos[c] = assign[c] * 128 + slot

    w = assign[core, n_loc]
    dst_loc = pos[core, n_loc] % 128
    src_pos = pos[src_core, src % NPC]
    src_loc = (src_core % 2) * NPC_PAD + src_pos

    bucket = ((core * WPC + w) * NCHUNK + k)
    cnt = np.bincount(bucket, minlength=NCORES * WPC * NCHUNK).reshape(
        NCORES, WPC, NCHUNK
    )
    T = np.maximum(1, -(-cnt // 128)).max(axis=0)  # [WPC, NCHUNK] int64

    # sort edges by bucket (stable, any order within bucket)
    order = np.argsort(bucket, kind="stable")
    src_loc_s = src_loc[order]
    dst_loc_s = dst_loc[order]
    bucket_s = bucket[order]
    starts = np.zeros(NCORES * WPC * NCHUNK + 1, dtype=np.int64)
    np.cumsum(np.bincount(bucket_s, minlength=NCORES * WPC * NCHUNK), out=starts[1:])

    tot_tiles = int(T.sum())
    tot_slots = tot_tiles * 128

    # per-(p,k): column base within phase msgs tile and call length
    phase_tiles = []  # [p] -> total tiles in phase
    call_info = []  # [p][k] = (idx_off_slots, n_slots, col_base)
    gcol = 0
    for p in range(NPHASE):
        ws = range(p * G_WIN, (p + 1) * G_WIN)
        info = []
        col = 0
        for kk in range(NCHUNK):
            n_t = int(sum(T[ww][kk] for ww in ws))
            info.append((gcol * 128, n_t * 128, col))
            col += n_t
            gcol += n_t
        call_info.append(info)
        phase_tiles.append(col)
    assert gcol == tot_tiles

    # map (w,k) -> global tile start
    tile_start = np.zeros((WPC, NCHUNK), dtype=np.int64)
    g = 0
    for p in range(NPHASE):
        for kk in range(NCHUNK):
            for ww in range(p * G_WIN, (p + 1) * G_WIN):
                tile_start[ww][kk] = g
                g += T[ww][kk]

    # build per-core slot arrays
    import ml_dtypes

    per_core = []
    for c in range(NCORES):
        idx_arr = np.zeros(tot_slots, dtype=np.int16)
        dl_arr = np.full(tot_slots, -1, dtype=np.int64)
        for ww in range(WPC):
            for kk in range(NCHUNK):
                b = (c * WPC + ww) * NCHUNK + kk
                s0, s1 = starts[b], starts[b + 1]
                n = s1 - s0
                o = tile_start[ww][kk] * 128
                cap = T[ww][kk] * 128
                assert n <= cap
                idx_arr[o : o + n] = src_loc_s[s0:s1]
                dl_arr[o : o + n] = dst_loc_s[s0:s1]
                if n < cap:  # pad with a valid idx (row 0 of chunk), dst -1
                    idx_arr[o + n : o + cap] = 0
        # wrap idx: position i -> [16r + i%16, i//16]
        idx_w = idx_arr.reshape(-1, 16).T  # [16, tot/16]
        idx_w = np.tile(idx_w, (8, 1))  # [128, tot/16]
        # per-slot dst-loc table for on-device S build: dl_t[p, 2g] =
        # dl_t[p, 2g+1] = dst loc of slot g*128+p (or -1 for pad slots, which
        # never match iota). Pairwise duplication gives the DVE is_equal a
        # stride-1 innermost dim (2x perf mode) instead of a pure broadcast.
        dl_t = np.repeat(
            dl_arr.reshape(tot_tiles, 128).T.astype(ml_dtypes.bfloat16), 2, axis=1
        )
        per_core.append((idx_w, np.ascontiguousarray(dl_t)))

    # per-core dinv arrays [128, WPC] (pad nodes -> 1.0), permuted to pos order
    def slice_arr(a):
        out = np.ones((NCORES, NPC_PAD), dtype=np.float32)
        av = a.reshape(NCORES, NPC)
        for c in range(NCORES):
            out[c, pos[c]] = av[c]
        return out.reshape(NCORES, WPC, 128).transpose(0, 2, 1).copy()

    dinv_t = slice_arr(dinv)
    # dinv in global (core, pos) order for the full u0 sweep: [128, GW_ALL]
    dinv_full = np.concatenate([dinv_t[c] for c in range(NCORES)], axis=1)

    return {
        "pos": pos,
        "T": T,
        "tot_tiles": tot_tiles,
        "phase_tiles": phase_tiles,
        "call_info": call_info,
        "tile_start": tile_start,
        "per_core": per_core,
        "dinv_t": dinv_t,
        "dinv2_t": slice_arr(dinv2),
        "dsqrt_t": slice_arr(dsqrt),
        "dinv_full": np.ascontiguousarray(dinv_full),
    }


# ---------------------------------------------------------------- bass build
def _build_nc(T, tot_tiles, phase_tiles, call_info, tile_start, reps=1):
    import concourse.bacc as bacc
    import concourse.mybir as mybir
    import concourse.tile as tile
    from concourse.library_config import mlp

    f32 = mybir.dt.float32
    bf16 = mybir.dt.bfloat16
    i16 = mybir.dt.int16

    tot16 = tot_tiles * 128 // 16

    nc = bacc.Bacc("TRN2", target_bir_lowering=False, debug=False,
                   num_devices=NCORES, num_swdge_queues=4)

    featT_in = nc.dram_tensor("featT_in", [F_IN, NPC_PAD], f32, kind="ExternalInput")
    featfullT_in = nc.dram_tensor(
        "featfullT_in", [F_IN, NTAB], bf16, kind="ExternalInput"
    )
    w_in = nc.dram_tensor("w_in", [F_IN, H], f32, kind="ExternalInput")
    b_in = nc.dram_tensor("b_in", [1, H], f32, kind="ExternalInput")
    iota_in = nc.dram_tensor("iota_in", [128, SBATCH * 128], bf16,
                             kind="ExternalInput")
    idx_in = nc.dram_tensor("idx_in", [128, tot16], i16, kind="ExternalInput")
    dl_in = nc.dram_tensor("dl_in", [128, 2 * tot_tiles], bf16,
                           kind="ExternalInput")
    dinv_in = nc.dram_tensor("dinv_in", [128, WPC], f32, kind="ExternalInput")
    dinv2_in = nc.dram_tensor("dinv2_in", [128, WPC], f32, kind="ExternalInput")
    dsqrt_in = nc.dram_tensor("dsqrt_in", [128, WPC], f32, kind="ExternalInput")
    dinvfull_in = nc.dram_tensor("dinvfull_in", [128, GW_ALL], f32,
                                 kind="ExternalInput")
    out = nc.dram_tensor("out", [NPC_PAD, 4 * H], f32, kind="ExternalOutput")

    SLAB = WPC * H  # 6272 free elems

    with tile.TileContext(nc) as tc:
        with (
            tc.tile_pool(name="dram", bufs=1, space="DRAM") as dram,
            tc.tile_pool(name="const", bufs=1) as const,
            tc.tile_pool(name="slabs", bufs=1) as slabs,
            tc.tile_pool(name="work", bufs=3) as work,
            tc.tile_pool(name="msgs_p", bufs=2) as msgs_pool,
            tc.tile_pool(name="psum", bufs=4, space="PSUM") as psum_pool,
        ):
            nc.gpsimd.load_library(mlp)

            # ---------- constants / metadata loads
            w_sb = const.tile([F_IN, H], f32)
            nc.sync.dma_start(out=w_sb[:], in_=w_in[:])
            w16 = const.tile([F_IN, H], bf16)
            nc.vector.tensor_copy(out=w16[:], in_=w_sb[:])
            b_sb = const.tile([1, H], f32)
            nc.sync.dma_start(out=b_sb[:], in_=b_in[:])
            b16 = const.tile([1, H], bf16)
            nc.vector.tensor_copy(out=b16[:], in_=b_sb[:])
            iota_sb = const.tile([128, SBATCH * 128], bf16)
            nc.sync.dma_start(out=iota_sb[:], in_=iota_in[:])
            dinv_sb = const.tile([128, WPC], f32)
            nc.sync.dma_start(out=dinv_sb[:], in_=dinv_in[:])
            dinv2_sb = const.tile([128, WPC], f32)
            nc.sync.dma_start(out=dinv2_sb[:], in_=dinv2_in[:])
            dsqrt_sb = const.tile([128, WPC], f32)
            nc.sync.dma_start(out=dsqrt_sb[:], in_=dsqrt_in[:])
            dinvfull_sb = const.tile([128, GW_ALL], f32)
            nc.sync.dma_start(out=dinvfull_sb[:], in_=dinvfull_in[:])
            ones_col = const.tile([1, 128], f32)
            nc.vector.memset(ones_col[:], 1.0)
            ones16 = const.tile([1, 128], bf16)
            nc.vector.memset(ones16[:], 1.0)

            slab_a = slabs.tile([128, SLAB], f32)  # u0 -> u2
            slab_b = slabs.tile([128, SLAB], f32)  # u1 -> u3

            saves = [
                dram.tile([128, SLAB], f32, name=f"save{kk}") for kk in range(2)
            ]
            ag_h = dram.tile([NPC_PAD, 2 * H], mybir.dt.bfloat16, name="agin")
            stage16 = slabs.tile([128, SLAB], mybir.dt.bfloat16, name="stg")

            for rep in range(reps):
                _emit_body(
                    nc, tc, mybir, rep, T, tot_tiles, phase_tiles, call_info,
                    tile_start, dram, work, msgs_pool, psum_pool, slab_a, slab_b,
                    saves, ag_h, featT_in, featfullT_in, out, w_sb, b_sb,
                    w16, b16, iota_sb, idx_in, dl_in, dinv_sb, dinv2_sb,
                    dsqrt_sb, dinvfull_sb, ones_col, ones16, stage16,
                )

    nc.compile()
    return nc


def _emit_body(
    nc, tc, mybir, rep, T, tot_tiles, phase_tiles, call_info, tile_start,
    dram, work, msgs_pool, psum_pool, slab_a, slab_b, saves, ag_h, featT_in,
    featfullT_in, out, w_sb, b_sb, w16, b16, iota_sb, idx_in, dl_in,
    dinv_sb, dinv2_sb, dsqrt_sb, dinvfull_sb, ones_col, ones16, stage16,
):
    import concourse.mybir as mybir

    f32 = mybir.dt.float32
    bf16 = mybir.dt.bfloat16
    i16 = mybir.dt.int16
    Alu = mybir.AluOpType
    SLAB = WPC * H
    max_ptiles = max(phase_tiles)

    tables = [
        dram.tile(
            [NTAB, 2 * H], mybir.dt.bfloat16,
            **({} if hop == 0 else {"addr_space": "Shared"}),
            name=f"tbl{rep}_{hop}",
        )
        for hop in range(NHOP)
    ]

    def launch_ag(hop_dst):
        nc.gpsimd.collective_compute(
            "AllGather",
            mybir.AluOpType.bypass,
            replica_groups=[list(range(NCORES))],
            ins=[ag_h.opt()],
            outs=[tables[hop_dst].opt()],
        )

    if True:  # keep indentation of original body
            # ---------- u0 table (ALL nodes, bf16) built locally; plus the
            # local f32 u0 slab for this core's own windows.
            GB = 8  # global windows per feature-stream batch
            with tc.tile_pool(name=f"featp{rep}", bufs=2) as featp:
                # full sweep in global (chunk-major) order -> tables[0]
                for g0 in range(0, GW_ALL, GB):
                    gb = min(GB, GW_ALL - g0)
                    fT = featp.tile([128, GB * 128], bf16, tag="fT", bufs=3)
                    nc.sync.dma_start(
                        out=fT[:, : gb * 128],
                        in_=featfullT_in[:, g0 * 128 : (g0 + gb) * 128],
                    )
                    u0st = featp.tile([128, GB, H], bf16, tag="u0st")
                    for j in range(gb):
                        gw = g0 + j
                        h_ps = psum_pool.tile([128, 128], f32, tag="ps", bufs=8)
                        nc.tensor.matmul(
                            out=h_ps[:, :H], lhsT=fT[:, j * 128 : (j + 1) * 128],
                            rhs=w16[:], start=True, stop=False,
                        )
                        nc.tensor.matmul(
                            out=h_ps[:, :H], lhsT=ones16[:], rhs=b16[:],
                            start=False, stop=True,
                        )
                        t2 = work.tile([128, H], f32, tag="t2", bufs=4)
                        nc.scalar.activation(
                            out=t2[:], in_=h_ps[:, :H],
                            func=mybir.ActivationFunctionType.Lrelu, alpha=0.01,
                        )
                        nc.vector.tensor_scalar(
                            out=u0st[:, j, :], in0=t2[:],
                            scalar1=dinvfull_sb[:, gw : gw + 1], scalar2=None,
                            op0=Alu.mult,
                        )
                    nc.sync.dma_start(
                        out=tables[0][g0 * 128 : (g0 + gb) * 128, :H].rearrange(
                            "(w p) h -> p w h", p=128
                        ),
                        in_=u0st[:, :gb, :],
                    )
                # local f32 slab for this core's own nodes
                for w0 in range(0, WPC, G_WIN):
                    gw = min(G_WIN, WPC - w0)
                    fTl = featp.tile([128, G_WIN * 128], f32, tag="fTl")
                    nc.sync.dma_start(
                        out=fTl[:, : gw * 128],
                        in_=featT_in[:, w0 * 128 : (w0 + gw) * 128],
                    )
                    for w in range(w0, w0 + gw):
                        h_ps = psum_pool.tile([128, 128], f32, tag="ps", bufs=8)
                        nc.tensor.matmul(
                            out=h_ps[:, :H],
                            lhsT=fTl[:, (w - w0) * 128 : (w - w0 + 1) * 128],
                            rhs=w_sb[:], start=True, stop=False,
                        )
                        nc.tensor.matmul(
                            out=h_ps[:, :H], lhsT=ones_col[:], rhs=b_sb[:],
                            start=False, stop=True,
                        )
                        t2 = work.tile([128, H], f32, tag="t2", bufs=4)
                        nc.scalar.activation(
                            out=t2[:], in_=h_ps[:, :H],
                            func=mybir.ActivationFunctionType.Lrelu, alpha=0.01,
                        )
                        nc.vector.tensor_scalar(
                            out=slab_a[:, w * H : (w + 1) * H], in0=t2[:],
                            scalar1=dinv_sb[:, w : w + 1], scalar2=None,
                            op0=Alu.mult,
                        )
            nc.sync.dma_start(out=saves[0][:], in_=slab_a[:])

            # ---------- hops
            cur, nxt = slab_a, slab_b
            for hop in range(NHOP):
                g = 0  # global tile counter
                for p in range(NPHASE):
                    ptiles = phase_tiles[p]
                    p_off16 = call_info[p][0][0] // 16  # phase idx col start
                    p_len16 = ptiles * 128 // 16
                    first_g_p = call_info[p][0][0] // 128
                    idxp = msgs_pool.tile(
                        [128, (max_ptiles * 128) // 16], i16, tag="idxp"
                    )
                    nc.sync.dma_start(
                        out=idxp[:, :p_len16],
                        in_=idx_in[:, p_off16 : p_off16 + p_len16],
                    )
                    # on-the-fly S build: load per-slot dst-locs (pairwise
                    # duplicated), compare against iota (SBATCH tiles per DVE
                    # op). The [stride 1, count 2] innermost dim on in1 keeps
                    # the op eligible for the DVE 2x perf mode.
                    dlp = msgs_pool.tile([128, 2 * max_ptiles], bf16, tag="dlp")
                    nc.sync.dma_start(
                        out=dlp[:, : 2 * ptiles],
                        in_=dl_in[:, 2 * first_g_p : 2 * (first_g_p + ptiles)],
                    )
                    s_built = msgs_pool.tile(
                        [128, max_ptiles * 128], bf16, tag="sb", bufs=1
                    )
                    for t0 in range(0, ptiles, SBATCH):
                        tb = min(SBATCH, ptiles - t0)
                        nc.vector.tensor_tensor(
                            out=s_built[:, t0 * 128 : (t0 + tb) * 128].rearrange(
                                "p (t jh jl) -> p t jh jl", jl=2, jh=64
                            ),
                            in0=iota_sb[:, : tb * 128].rearrange(
                                "p (t jh jl) -> p t jh jl", jl=2, jh=64
                            ),
                            in1=dlp[:, 2 * t0 : 2 * (t0 + tb)].rearrange(
                                "p (t o jl) -> p t o jl", jl=2, o=1
                            ).to_broadcast([128, tb, 64, 2]),
                            op=Alu.is_equal,
                        )
                    msgs16 = msgs_pool.tile(
                        [128, max_ptiles, 2 * H], bf16, tag="msgs16"
                    )
                    GCAP = 8192  # max idxs per dma_gather (desc-ring capacity)
                    for kk in range(NCHUNK):
                        off_sl, n_sl, col = call_info[p][kk]
                        for o in range(0, n_sl, GCAP):
                            ln = min(GCAP, n_sl - o)
                            c0 = col + o // 128
                            i0 = (off_sl + o) // 16 - p_off16
                            nc.gpsimd.dma_gather(
                                msgs16[:, c0 : c0 + ln // 128, :],
                                tables[hop][CH_PAD * kk : CH_PAD * (kk + 1), :],
                                idxp[:, i0 : i0 + ln // 16],
                                ln,
                                ln,
                                2 * H,
                                single_packet=False,
                                queue_num=kk,
                            )
                    first_g = g
                    # matmuls per window
                    for ww in range(p * G_WIN, (p + 1) * G_WIN):
                        agg_ps = psum_pool.tile([128, 128], f32, tag="ps", bufs=8)
                        n_mm = int(sum(T[ww][kk] for kk in range(NCHUNK)))
                        mm_i = 0
                        for kk in range(NCHUNK):
                            _, _, col = call_info[p][kk]
                            cbase = col + int(
                                sum(T[w2][kk] for w2 in range(p * G_WIN, ww))
                            )
                            for t in range(int(T[ww][kk])):
                                # global tile index in host (p,k,w,t) order
                                lg = int(tile_start[ww][kk]) + t - first_g
                                nc.tensor.matmul(
                                    out=agg_ps[:, :H],
                                    lhsT=s_built[:, lg * 128 : (lg + 1) * 128],
                                    rhs=msgs16[:, cbase + t, :H],
                                    start=(mm_i == 0),
                                    stop=(mm_i == n_mm - 1),
                                )
                                mm_i += 1
                        g += n_mm
                        # u' = u - dinv2 * agg  (scale on ACT, subtract on DVE)
                        tscl = work.tile([128, H], f32, tag="tscl", bufs=4)
                        nc.scalar.activation(
                            out=tscl[:], in_=agg_ps[:, :H],
                            func=mybir.ActivationFunctionType.Copy,
                            scale=dinv2_sb[:, ww : ww + 1],
                        )
                        nc.vector.tensor_tensor(
                            out=nxt[:, ww * H : (ww + 1) * H],
                            in0=cur[:, ww * H : (ww + 1) * H],
                            in1=tscl[:],
                            op=Alu.subtract,
                        )
                    if hop < NHOP - 1:
                        # pre-cast this phase's u slice to bf16 and stage it
                        # into ag_h right away, so the hop boundary pays
                        # nothing before the AllGather launches
                        cs2 = slice(p * G_WIN * H, (p + 1) * G_WIN * H)
                        nc.vector.tensor_copy(
                            out=stage16[:, cs2], in_=nxt[:, cs2]
                        )
                        w0p = p * G_WIN
                        nc.sync.dma_start(
                            out=ag_h[
                                w0p * 128 : (w0p + G_WIN) * 128, :H
                            ].rearrange("(w p) h -> p w h", p=128),
                            in_=stage16[:, cs2].rearrange(
                                "p (w h) -> p w h", h=H
                            ),
                        )
                    if hop == NHOP - 1:
                        # combine this phase's windows now (u3 slice just
                        # written; u0/u1 from DRAM, u2 = cur) -- hides the
                        # epilogue under the gather-bound last hop
                        w0 = p * G_WIN
                        cs = slice(w0 * H, (w0 + G_WIN) * H)
                        u0c = work.tile([128, G_WIN * H], f32, tag="u0c",
                                        bufs=2)
                        nc.sync.dma_start(out=u0c[:], in_=saves[0][:, cs])
                        u1c = work.tile([128, G_WIN * H], f32, tag="u1c",
                                        bufs=2)
                        nc.sync.dma_start(out=u1c[:], in_=saves[1][:, cs])
                        us = [u0c[:], u1c[:], cur[:, cs], nxt[:, cs]]
                        out_st = work.tile([128, G_WIN, 4 * H], f32,
                                           tag="outst", bufs=2)
                        for i in range(4):
                            acc = work.tile([128, G_WIN * H], f32, tag="acc",
                                            bufs=2)
                            a = acc[:]
                            ks = [kk for kk in range(4)
                                  if THETAS[i][kk] != 0.0]
                            nc.scalar.activation(
                                out=a, in_=us[ks[0]],
                                func=mybir.ActivationFunctionType.Copy,
                                scale=float(THETAS[i][ks[0]]),
                            )
                            for kk in ks[1:]:
                                tmp = work.tile([128, G_WIN * H], f32,
                                                tag="ctmp", bufs=2)
                                nc.scalar.activation(
                                    out=tmp[:], in_=us[kk],
                                    func=mybir.ActivationFunctionType.Copy,
                                    scale=float(THETAS[i][kk]),
                                )
                                nc.vector.tensor_tensor(
                                    out=a, in0=a, in1=tmp[:], op=Alu.add
                                )
                            nc.vector.tensor_tensor(
                                out=out_st[:, :, i * H : (i + 1) * H],
                                in0=a.rearrange("p (w h) -> p w h", h=H),
                                in1=dsqrt_sb[:, w0 : w0 + G_WIN].to_broadcast(
                                    [128, G_WIN, H]
                                ),
                                op=Alu.mult,
                            )
                        nc.sync.dma_start(
                            out=out[
                                w0 * 128 : (w0 + G_WIN) * 128, :
                            ].rearrange("(w p) h -> p w h", p=128),
                            in_=out_st[:],
                        )
                assert g == sum(phase_tiles[:NPHASE])
                if hop < NHOP - 1:
                    launch_ag(hop + 1)
                if hop == 0:
                    nc.sync.dma_start(out=saves[1][:], in_=nxt[:])  # save u1
                cur, nxt = nxt, cur


# ---------------------------------------------------------------- runner
def _make_runner(nc, in_maps, n_cores):
    import jax
    import numpy as np
    from jax.sharding import Mesh, NamedSharding, PartitionSpec
    from jax.experimental.shard_map import shard_map

    import concourse.mybir as mybir
    from concourse import bass2jax

    bass2jax.install_neuronx_cc_hook()
    partition_name = nc.partition_id_tensor.name if nc.partition_id_tensor else None
    in_names, out_names, out_avals, zero_outs = [], [], [], []
    for alloc in nc.m.functions[0].allocations:
        if not isinstance(alloc, mybir.MemoryLocationSet):
            continue
        name = alloc.memorylocations[0].name
        if alloc.kind == "ExternalInput":
            if name != partition_name:
                in_names.append(name)
        elif alloc.kind == "ExternalOutput":
            out_names.append(name)
            shape = tuple(alloc.tensor_shape)
            dtype = mybir.dt.np(alloc.dtype)
            out_avals.append(jax.core.ShapedArray(shape, dtype))
            zero_outs.append(np.zeros(shape, dtype))
    n_params = len(in_names)
    all_in_names = list(in_names) + list(out_names)
    if partition_name is not None:
        all_in_names.append(partition_name)

    def _body(*args):
        operands = list(args)
        if partition_name is not None:
            operands.append(bass2jax.partition_id_tensor())
        outs = bass2jax._bass_exec_p.bind(
            *operands,
            out_avals=tuple(out_avals),
            in_names=tuple(all_in_names),
            out_names=tuple(out_names),
            lowering_input_output_aliases=(),
            sim_require_finite=True,
            sim_require_nnan=True,
            nc=nc,
        )
        return tuple(outs)

    devices = jax.devices()[:n_cores]
    mesh = Mesh(np.asarray(devices), ("core",))
    n_ops = n_params + len(out_names)
    sharded = jax.jit(
        shard_map(
            _body,
            mesh=mesh,
            in_specs=(PartitionSpec("core"),) * n_ops,
            out_specs=(PartitionSpec("core"),) * len(out_names),
            check_rep=False,
        ),
        keep_unused=True,
    )
    sh = NamedSharding(mesh, PartitionSpec("core"))
    concat_in = [
        jax.device_put(
            np.concatenate([np.asarray(in_maps[c][nm]) for c in range(n_cores)], 0),
            sh,
        )
        for nm in in_names
    ]
    concat_zeros = [
        jax.device_put(np.zeros((n_cores * z.shape[0], *z.shape[1:]), z.dtype), sh)
        for z in zero_outs
    ]
    args = concat_in + concat_zeros

    def run():
        return sharded(*args)

    return run, out_names, out_avals


_CACHE = {}


def _get_built(edge_index_bytes_key, edge_index):
    if edge_index_bytes_key not in _CACHE:
        prep = _prep(edge_index)
        nc = _build_nc(
            prep["T"],
            prep["tot_tiles"],
            prep["phase_tiles"],
            prep["call_info"],
            prep["tile_start"],
        )
        _CACHE[edge_index_bytes_key] = (prep, nc)
    return _CACHE[edge_index_bytes_key]


def _make_in_maps(prep, inputs):
    import ml_dtypes

    feature = np.asarray(inputs["feature"], dtype=np.float32)
    W = np.asarray(inputs["W"], dtype=np.float32)
    b = np.asarray(inputs["b"], dtype=np.float32)

    b2 = b.reshape(1, H)
    iota = np.tile(np.arange(128, dtype=np.float32), SBATCH).reshape(1, -1)
    iota = np.repeat(iota, 128, axis=0).astype(ml_dtypes.bfloat16)

    pos = prep["pos"]
    feat_pad = np.zeros((NCORES, NPC_PAD, F_IN), dtype=np.float32)
    fv = feature.reshape(NCORES, NPC, F_IN)
    for c in range(NCORES):
        feat_pad[c, pos[c], :] = fv[c]
    # transposed per-core features [F, NPC_PAD] f32
    featT = np.ascontiguousarray(feat_pad.transpose(0, 2, 1))
    # transposed full features (global core-major order), bf16, same for all
    featfullT = np.ascontiguousarray(
        feat_pad.reshape(NCORES * NPC_PAD, F_IN).T.astype(ml_dtypes.bfloat16)
    )

    in_maps = []
    for c in range(NCORES):
        idx_w, dl_t = prep["per_core"][c]
        in_maps.append(
            {
                "featT_in": featT[c],
                "featfullT_in": featfullT,
                "w_in": W,
                "b_in": b2,
                "iota_in": iota,
                "idx_in": idx_w,
                "dl_in": dl_t,
                "dinv_in": prep["dinv_t"][c],
                "dinv2_in": prep["dinv2_t"][c],
                "dsqrt_in": prep["dsqrt_t"][c],
                "dinvfull_in": prep["dinv_full"],
            }
        )
    return in_maps


def kernel(feature, edge_index, W, b):
    import jax

    edge_index = np.asarray(edge_index, dtype=np.int32)
    key = hash(edge_index.tobytes())
    prep, nc = _get_built(key, edge_index)
    in_maps = _make_in_maps(prep, {"feature": feature, "W": W, "b": b})

    run, out_names, out_avals = _make_runner(nc, in_maps, NCORES)
    outs = jax.block_until_ready(run())
    oi = out_names.index("out")
    full = np.asarray(outs[oi]).reshape(NCORES, NPC_PAD, 4 * H)
    pos = prep["pos"]
    res = np.empty((NCORES, NPC, 4 * H), np.float32)
    for c in range(NCORES):
        res[c] = full[c, pos[c], :]
    return res.reshape(N, 4 * H)


if __name__ == "__main__":
    rng = np.random.default_rng(0)
    feature = rng.standard_normal((N, F_IN), dtype=np.float32)
    edge_index = rng.integers(0, N, (2, E)).astype(np.int32)
    W = (rng.standard_normal((F_IN, H)) * 0.05).astype(np.float32)
    b = (rng.standard_normal((H,)) * 0.05).astype(np.float32)
    out = kernel(feature=feature, edge_index=edge_index, W=W, b=b)
    print(out.shape, out.dtype, float(np.abs(out).mean()))


# revision 24
# speedup vs baseline: 1.3475x; 1.0186x over previous
"""BWGNN (Beta Wavelet GNN) Trainium2 kernel, 8-way SPMD.

Math (reference.py): deg = out-degree(src) clamped >=1; Dinv = deg^-1/2;
h = leaky_relu(feature @ W + b); L feat = feat - Dinv*segsum_dst(Dinv[src]*feat[src]);
out = concat_i sum_k THETA[i][k] L^k h.

We iterate on u_k = Dinv * L^k h:
    u_{k+1} = u_k - Dinv^2 * segsum_dst(u_k[src])
    out_i   = (sum_k THETA[i][k] u_k) * deg^{1/2}

Distribution: nodes dst-sharded over 8 cores (12500 + pad -> 12544 rows/core).
Full u-table (bf16, 256B rows = 64 values + 64 pad cols) lives in each core's
HBM; tables 1,2 are refreshed per hop by an intra-chip AllGather of the
per-core bf16 slices.  Table 0 (u0) is built LOCALLY on every core from a
replicated transposed bf16 copy of the full feature matrix -- no AllGather
for hop 1.

Key performance structure:
 - Per-edge u[src] rows are pulled by dma_gather (SWDGE, int16 local idx)
   spread across all 4 SWDGE queues (queue_num=chunk) so descriptor
   generation runs on all 4 Q7 core-pairs concurrently.
 - Nodes are assigned to 128-node dst-windows by a host-side balancing pass
   (greedy + swap repair) so nearly every (window, src-chunk) bucket fits in
   4 tiles of 128 edges (~2% slot padding).
 - The segment-sum one-hot S matrices are built ON THE FLY by the DVE
   (iota is_equal against a per-slot dst-loc table, 8 tiles per op) --
   replaces streaming 52MB/hop of host-built one-hots from HBM.
 - PE matmuls (bf16 lhsT=S, rhs=gathered msgs) accumulate each window in
   PSUM; the PSUM scale by Dinv^2 runs on the Activation engine; DVE does
   the S build, subtract, u0 activation scale and the final combine.
 - Features arrive pre-transposed ([F, nodes]) so the u0 stage needs no PE
   transposes; outputs of the 4 wavelets are staged and stored with one
   1KB-per-row DMA per phase.
"""

import math
import os
import sys

sys.path.insert(0, "/opt/trn_rl_repo")

import numpy as np

# ---------------------------------------------------------------- constants
N = 100000
E = 1600000
F_IN = 128
H = 64
NCORES = 8
NPC = 12500          # nodes per core
WPC = 98             # windows (128-node groups) per core
NPC_PAD = WPC * 128  # 12544
NCHUNK = 4
CH_NODES = 25000     # original nodes per chunk
CH_PAD = 2 * NPC_PAD  # 25088 padded rows per chunk
NTAB = NCORES * NPC_PAD  # 100352
GW_ALL = NTAB // 128     # 784 global windows
G_WIN = 7            # windows per phase
NPHASE = WPC // G_WIN  # 14
SBATCH = 8           # S tiles built per DVE instruction (256-wide)
NHOP = 3


def _calculate_theta2(d):
    thetas = []
    for i in range(d):
        c1 = np.zeros(i + 1)
        c1[i] = 0.5 ** i
        c2 = np.array([math.comb(d - i, j) * (-0.5) ** j for j in range(d - i + 1)])
        c = np.convolve(c1, c2)
        B = math.factorial(i) * math.factorial(d - i) / math.factorial(d + 1)
        c = c / (2.0 * B)
        thetas.append([float(c[j]) for j in range(d)])
    return thetas


THETAS = _calculate_theta2(4)  # [4][4], theta[i][k] weight of L^k h in output i


# ---------------------------------------------------------------- host prep
def _balance_windows(indeg, half, nwin, over_idx):
    """Assign nodes to `nwin` windows keeping per-(window,chunk) counts under
    512 except the overflow windows in `over_idx` (640). Greedy + swap repair.
    indeg: [NCORES, NPC, NCHUNK]; half: [NCORES, NPC] bool mask of nodes to
    place. Returns assign [NCORES, NPC] -> window in [0, nwin)."""
    WPC_, NCHUNK_ = nwin, NCHUNK
    CAP = np.full((WPC_, NCHUNK_), 512, np.int64)
    CAP[list(over_idx), :] = 640
    assign = np.full((NCORES, NPC), -1, np.int32)
    for c in range(NCORES):
        nodes_c = np.where(half[c])[0]
        d_all = indeg[c].astype(np.int64)
        d_vec = d_all
        order = nodes_c[np.argsort(-d_all[nodes_c].sum(1), kind="stable")]
        L = np.zeros((WPC_, NCHUNK_), np.int64)
        counts = np.zeros(WPC_, np.int64)
        for n in order:
            dv = d_vec[n]
            excess = np.maximum(L + dv - CAP, 0).sum(axis=1)
            tot = (L + dv).max(axis=1)
            score = excess * 100000 + tot
            score[counts >= 128] = 1 << 60
            ww = int(np.argmin(score))
            L[ww] += dv
            counts[ww] += 1
            assign[c, n] = ww
        for _sweep in range(20):
            L = np.zeros((WPC_, NCHUNK_), np.int64)
            for kk in range(NCHUNK):
                np.add.at(L[:, kk], assign[c][nodes_c], d_vec[nodes_c, kk])
            over = np.argwhere(L > CAP)
            if len(over) == 0:
                break
            win_nodes = [np.where(assign[c] == ww)[0] for ww in range(WPC_)]
            for ww, kk in over:
                while L[ww, kk] > CAP[ww, kk]:
                    nodes = win_nodes[ww]
                    nodes = nodes[d_vec[nodes, kk] > 0]
                    if len(nodes) == 0:
                        break
                    n = nodes[np.argmax(d_vec[nodes, kk])]
                    dn = d_vec[n]
                    done = False
                    for w2 in np.argsort(L[:, kk]):
                        if w2 == ww:
                            continue
                        cand = win_nodes[w2]
                        if len(cand) == 0:
                            continue
                        dm = d_vec[cand]
                        ok = ((L[w2] + dn - dm) <= CAP[w2]).all(1) & \
                             (dm[:, kk] < dn[kk])
                        ok &= ((L[ww] - dn + dm) <= CAP[ww]).all(1) | (
                            ((L[ww] - dn + dm) < L[ww]).any(1)
                            & (dm[:, kk] < dn[kk])
                        )
                        if ok.any():
                            m = cand[np.argmax(ok)]
                            assign[c, n], assign[c, m] = w2, ww
                            L[ww] += d_vec[m] - dn
                            L[w2] += dn - d_vec[m]
                            win_nodes[ww] = np.where(assign[c] == ww)[0]
                            win_nodes[w2] = np.where(assign[c] == w2)[0]
                            done = True
                            break
                    if not done:
                        break
    return assign


def _prep(edge_index: np.ndarray):
    """Bucket edges, build per-core gather-index / dst-loc arrays and the
    static tile-count table T[w][k] (shared by all cores)."""
    src = edge_index[0].astype(np.int64)
    dst = edge_index[1].astype(np.int64)

    deg = np.bincount(src, minlength=N).astype(np.float32)
    dinv = np.maximum(deg, np.float32(1.0)) ** np.float32(-0.5)
    dinv2 = dinv * dinv
    dsqrt = np.float32(1.0) / dinv  # = max(deg,1)^0.5

    core = dst // NPC
    n_loc = dst % NPC
    src_core = src // NPC
    k = src // CH_NODES

    indeg = np.zeros((NCORES, NPC, NCHUNK), np.int32)
    np.add.at(indeg, (core, n_loc, k), 1)
    # spread the 4 overflow windows across phases so no phase is inflated
    assign = _balance_windows(
        indeg, np.ones((NCORES, NPC), bool), WPC, (0, 28, 56, 84)
    ).astype(np.int64)

    # pos[c][n] = window*128 + slot (slot = rank within window, <128)
    pos = np.zeros((NCORES, NPC), np.int64)
    for c in range(NCORES):
        order = np.argsort(assign[c], kind="stable")
        slot = np.zeros(NPC, np.int64)
        ww = assign[c][order]
        first = np.searchsorted(ww, np.arange(WPC), side="left")
        slot[order] = np.arange(NPC) - first[ww]
        pos[c] = assign[c] * 128 + slot

    w = assign[core, n_loc]
    dst_loc = pos[core, n_loc] % 128
    src_pos = pos[src_core, src % NPC]
    src_loc = (src_core % 2) * NPC_PAD + src_pos
    src_row = src_loc // 2          # packed 2-nodes-per-256B-row table row
    src_par = src_loc % 2           # which half of the row

    bucket = ((core * WPC + w) * NCHUNK + k)
    cnt = np.bincount(bucket, minlength=NCORES * WPC * NCHUNK).reshape(
        NCORES, WPC, NCHUNK
    )
    T = np.maximum(1, -(-cnt // 128)).max(axis=0)  # [WPC, NCHUNK] int64

    # sort edges by bucket (stable, any order within bucket)
    order = np.argsort(bucket, kind="stable")
    src_loc_s = src_row[order]
    dst_loc_s = dst_loc[order] + 128 * src_par[order]  # parity in bit 7
    bucket_s = bucket[order]
    starts = np.zeros(NCORES * WPC * NCHUNK + 1, dtype=np.int64)
    np.cumsum(np.bincount(bucket_s, minlength=NCORES * WPC * NCHUNK), out=starts[1:])

    tot_tiles = int(T.sum())
    tot_slots = tot_tiles * 128

    # per-(p,k): column base within phase msgs tile and call length
    phase_tiles = []  # [p] -> total tiles in phase
    call_info = []  # [p][k] = (idx_off_slots, n_slots, col_base)
    gcol = 0
    for p in range(NPHASE):
        ws = range(p * G_WIN, (p + 1) * G_WIN)
        info = []
        col = 0
        for kk in range(NCHUNK):
            n_t = int(sum(T[ww][kk] for ww in ws))
            info.append((gcol * 128, n_t * 128, col))
            col += n_t
            gcol += n_t
        call_info.append(info)
        phase_tiles.append(col)
    assert gcol == tot_tiles

    # map (w,k) -> global tile start
    tile_start = np.zeros((WPC, NCHUNK), dtype=np.int64)
    g = 0
    for p in range(NPHASE):
        for kk in range(NCHUNK):
            for ww in range(p * G_WIN, (p + 1) * G_WIN):
                tile_start[ww][kk] = g
                g += T[ww][kk]

    # build per-core slot arrays
    import ml_dtypes

    per_core = []
    for c in range(NCORES):
        idx_arr = np.zeros(tot_slots, dtype=np.int16)
        dl_arr = np.full(tot_slots, -1, dtype=np.int64)
        for ww in range(WPC):
            for kk in range(NCHUNK):
                b = (c * WPC + ww) * NCHUNK + kk
                s0, s1 = starts[b], starts[b + 1]
                n = s1 - s0
                o = tile_start[ww][kk] * 128
                cap = T[ww][kk] * 128
                assert n <= cap
                idx_arr[o : o + n] = src_loc_s[s0:s1]
                dl_arr[o : o + n] = dst_loc_s[s0:s1]
                if n < cap:  # pad with a valid idx (row 0 of chunk), dst -1
                    idx_arr[o + n : o + cap] = 0
        # wrap idx: position i -> [16r + i%16, i//16]
        idx_w = idx_arr.reshape(-1, 16).T  # [16, tot/16]
        idx_w = np.tile(idx_w, (8, 1))  # [128, tot/16]
        # per-slot dst-loc table for on-device S build: dl_t[p, 2g] =
        # dl_t[p, 2g+1] = dst loc of slot g*128+p (or -1 for pad slots, which
        # never match iota). Pairwise duplication gives the DVE is_equal a
        # stride-1 innermost dim (2x perf mode) instead of a pure broadcast.
        dl_t = np.repeat(
            dl_arr.reshape(tot_tiles, 128).T.astype(ml_dtypes.bfloat16), 2, axis=1
        )
        per_core.append((idx_w, np.ascontiguousarray(dl_t)))

    # per-core dinv arrays [128, WPC] (pad nodes -> 1.0), permuted to pos order
    def slice_arr(a):
        out = np.ones((NCORES, NPC_PAD), dtype=np.float32)
        av = a.reshape(NCORES, NPC)
        for c in range(NCORES):
            out[c, pos[c]] = av[c]
        return out.reshape(NCORES, WPC, 128).transpose(0, 2, 1).copy()

    dinv_t = slice_arr(dinv)
    # dinv in global (core, pos) order for the full u0 sweep: [128, GW_ALL]
    dinv_full = np.concatenate([dinv_t[c] for c in range(NCORES)], axis=1)

    return {
        "pos": pos,
        "T": T,
        "tot_tiles": tot_tiles,
        "phase_tiles": phase_tiles,
        "call_info": call_info,
        "tile_start": tile_start,
        "per_core": per_core,
        "dinv_t": dinv_t,
        "dinv2_t": slice_arr(dinv2),
        "dsqrt_t": slice_arr(dsqrt),
        "dinv_full": np.ascontiguousarray(dinv_full),
    }


# ---------------------------------------------------------------- bass build
def _build_nc(T, tot_tiles, phase_tiles, call_info, tile_start, reps=1):
    import concourse.bacc as bacc
    import concourse.mybir as mybir
    import concourse.tile as tile
    from concourse.library_config import mlp

    f32 = mybir.dt.float32
    bf16 = mybir.dt.bfloat16
    i16 = mybir.dt.int16

    tot16 = tot_tiles * 128 // 16

    nc = bacc.Bacc("TRN2", target_bir_lowering=False, debug=False,
                   num_devices=NCORES, num_swdge_queues=4)

    featT_in = nc.dram_tensor("featT_in", [F_IN, NPC_PAD], f32, kind="ExternalInput")
    featfullT_in = nc.dram_tensor(
        "featfullT_in", [F_IN, NTAB], bf16, kind="ExternalInput"
    )
    w_in = nc.dram_tensor("w_in", [F_IN, H], f32, kind="ExternalInput")
    b_in = nc.dram_tensor("b_in", [1, H], f32, kind="ExternalInput")
    iota_in = nc.dram_tensor("iota_in", [128, SBATCH * 256], bf16,
                             kind="ExternalInput")
    idx_in = nc.dram_tensor("idx_in", [128, tot16], i16, kind="ExternalInput")
    dl_in = nc.dram_tensor("dl_in", [128, 2 * tot_tiles], bf16,
                           kind="ExternalInput")
    dinv_in = nc.dram_tensor("dinv_in", [128, WPC], f32, kind="ExternalInput")
    dinv2_in = nc.dram_tensor("dinv2_in", [128, WPC], f32, kind="ExternalInput")
    dsqrt_in = nc.dram_tensor("dsqrt_in", [128, WPC], f32, kind="ExternalInput")
    dinvfull_in = nc.dram_tensor("dinvfull_in", [128, GW_ALL], f32,
                                 kind="ExternalInput")
    out = nc.dram_tensor("out", [NPC_PAD, 4 * H], f32, kind="ExternalOutput")

    SLAB = WPC * H  # 6272 free elems

    with tile.TileContext(nc) as tc:
        with (
            tc.tile_pool(name="dram", bufs=1, space="DRAM") as dram,
            tc.tile_pool(name="const", bufs=1) as const,
            tc.tile_pool(name="slabs", bufs=1) as slabs,
            tc.tile_pool(name="work", bufs=3) as work,
            tc.tile_pool(name="msgs_p", bufs=2) as msgs_pool,
            tc.tile_pool(name="psum", bufs=4, space="PSUM") as psum_pool,
        ):
            nc.gpsimd.load_library(mlp)

            # ---------- constants / metadata loads
            w_sb = const.tile([F_IN, H], f32)
            nc.sync.dma_start(out=w_sb[:], in_=w_in[:])
            w16 = const.tile([F_IN, H], bf16)
            nc.vector.tensor_copy(out=w16[:], in_=w_sb[:])
            b_sb = const.tile([1, H], f32)
            nc.sync.dma_start(out=b_sb[:], in_=b_in[:])
            b16 = const.tile([1, H], bf16)
            nc.vector.tensor_copy(out=b16[:], in_=b_sb[:])
            iota_sb = const.tile([128, SBATCH * 256], bf16)
            nc.sync.dma_start(out=iota_sb[:], in_=iota_in[:])
            dinv_sb = const.tile([128, WPC], f32)
            nc.sync.dma_start(out=dinv_sb[:], in_=dinv_in[:])
            dinv2_sb = const.tile([128, WPC], f32)
            nc.sync.dma_start(out=dinv2_sb[:], in_=dinv2_in[:])
            dsqrt_sb = const.tile([128, WPC], f32)
            nc.sync.dma_start(out=dsqrt_sb[:], in_=dsqrt_in[:])
            dinvfull_sb = const.tile([128, GW_ALL], f32)
            nc.sync.dma_start(out=dinvfull_sb[:], in_=dinvfull_in[:])
            ones_col = const.tile([1, 128], f32)
            nc.vector.memset(ones_col[:], 1.0)
            ones16 = const.tile([1, 128], bf16)
            nc.vector.memset(ones16[:], 1.0)

            slab_a = slabs.tile([128, SLAB], bf16)  # u0 -> u2
            slab_b = slabs.tile([128, SLAB], bf16)  # u1 -> u3

            saves = [
                dram.tile([128, SLAB], bf16, name=f"save{kk}")
                for kk in range(2)
            ]
            ag_h = dram.tile([NPC_PAD, H], mybir.dt.bfloat16, name="agin")

            for rep in range(reps):
                _emit_body(
                    nc, tc, mybir, rep, T, tot_tiles, phase_tiles, call_info,
                    tile_start, dram, work, msgs_pool, psum_pool, slab_a, slab_b,
                    saves, ag_h, featT_in, featfullT_in, out, w_sb, b_sb,
                    w16, b16, iota_sb, idx_in, dl_in, dinv_sb, dinv2_sb,
                    dsqrt_sb, dinvfull_sb, ones_col, ones16,
                )

    nc.compile()
    return nc


def _emit_body(
    nc, tc, mybir, rep, T, tot_tiles, phase_tiles, call_info, tile_start,
    dram, work, msgs_pool, psum_pool, slab_a, slab_b, saves, ag_h, featT_in,
    featfullT_in, out, w_sb, b_sb, w16, b16, iota_sb, idx_in, dl_in,
    dinv_sb, dinv2_sb, dsqrt_sb, dinvfull_sb, ones_col, ones16,
):
    import concourse.mybir as mybir

    f32 = mybir.dt.float32
    bf16 = mybir.dt.bfloat16
    i16 = mybir.dt.int16
    Alu = mybir.AluOpType
    SLAB = WPC * H
    max_ptiles = max(phase_tiles)

    tables = [
        dram.tile(
            [NTAB, H], mybir.dt.bfloat16,
            **({} if hop == 0 else {"addr_space": "Shared"}),
            name=f"tbl{rep}_{hop}",
        )
        for hop in range(NHOP)
    ]

    def launch_ag(hop_dst):
        nc.gpsimd.collective_compute(
            "AllGather",
            mybir.AluOpType.bypass,
            replica_groups=[list(range(NCORES))],
            ins=[ag_h.opt()],
            outs=[tables[hop_dst].opt()],
        )

    if True:  # keep indentation of original body
            # ---------- u0 table (ALL nodes, bf16) built locally; plus the
            # local f32 u0 slab for this core's own windows.
            GB = 8  # global windows per feature-stream batch
            with tc.tile_pool(name=f"featp{rep}", bufs=2) as featp:
                # full sweep in global (chunk-major) order -> tables[0]
                for g0 in range(0, GW_ALL, GB):
                    gb = min(GB, GW_ALL - g0)
                    fT = featp.tile([128, GB * 128], bf16, tag="fT", bufs=3)
                    nc.sync.dma_start(
                        out=fT[:, : gb * 128],
                        in_=featfullT_in[:, g0 * 128 : (g0 + gb) * 128],
                    )
                    u0st = featp.tile([128, GB, H], bf16, tag="u0st")
                    for j in range(gb):
                        gw = g0 + j
                        h_ps = psum_pool.tile([128, 128], f32, tag="ps", bufs=8)
                        nc.tensor.matmul(
                            out=h_ps[:, :H], lhsT=fT[:, j * 128 : (j + 1) * 128],
                            rhs=w16[:], start=True, stop=False,
                        )
                        nc.tensor.matmul(
                            out=h_ps[:, :H], lhsT=ones16[:], rhs=b16[:],
                            start=False, stop=True,
                        )
                        t2 = work.tile([128, H], f32, tag="t2", bufs=4)
                        nc.scalar.activation(
                            out=t2[:], in_=h_ps[:, :H],
                            func=mybir.ActivationFunctionType.Lrelu, alpha=0.01,
                        )
                        nc.vector.tensor_scalar(
                            out=u0st[:, j, :], in0=t2[:],
                            scalar1=dinvfull_sb[:, gw : gw + 1], scalar2=None,
                            op0=Alu.mult,
                        )
                    nc.sync.dma_start(
                        out=tables[0][g0 * 128 : (g0 + gb) * 128, :H].rearrange(
                            "(w p) h -> p w h", p=128
                        ),
                        in_=u0st[:, :gb, :],
                    )
                # local f32 slab for this core's own nodes
                for w0 in range(0, WPC, G_WIN):
                    gw = min(G_WIN, WPC - w0)
                    fTl = featp.tile([128, G_WIN * 128], f32, tag="fTl")
                    nc.sync.dma_start(
                        out=fTl[:, : gw * 128],
                        in_=featT_in[:, w0 * 128 : (w0 + gw) * 128],
                    )
                    for w in range(w0, w0 + gw):
                        h_ps = psum_pool.tile([128, 128], f32, tag="ps", bufs=8)
                        nc.tensor.matmul(
                            out=h_ps[:, :H],
                            lhsT=fTl[:, (w - w0) * 128 : (w - w0 + 1) * 128],
                            rhs=w_sb[:], start=True, stop=False,
                        )
                        nc.tensor.matmul(
                            out=h_ps[:, :H], lhsT=ones_col[:], rhs=b_sb[:],
                            start=False, stop=True,
                        )
                        t2 = work.tile([128, H], f32, tag="t2", bufs=4)
                        nc.scalar.activation(
                            out=t2[:], in_=h_ps[:, :H],
                            func=mybir.ActivationFunctionType.Lrelu, alpha=0.01,
                        )
                        nc.vector.tensor_scalar(
                            out=slab_a[:, w * H : (w + 1) * H], in0=t2[:],
                            scalar1=dinv_sb[:, w : w + 1], scalar2=None,
                            op0=Alu.mult,
                        )
            nc.sync.dma_start(out=saves[0][:], in_=slab_a[:])

            # ---------- hops
            cur, nxt = slab_a, slab_b
            for hop in range(NHOP):
                g = 0  # global tile counter
                for p in range(NPHASE):
                    ptiles = phase_tiles[p]
                    p_off16 = call_info[p][0][0] // 16  # phase idx col start
                    p_len16 = ptiles * 128 // 16
                    first_g_p = call_info[p][0][0] // 128
                    idxp = msgs_pool.tile(
                        [128, (max_ptiles * 128) // 16], i16, tag="idxp"
                    )
                    nc.sync.dma_start(
                        out=idxp[:, :p_len16],
                        in_=idx_in[:, p_off16 : p_off16 + p_len16],
                    )
                    # on-the-fly S build: load per-slot dst-locs (pairwise
                    # duplicated), compare against iota (SBATCH tiles per DVE
                    # op). The [stride 1, count 2] innermost dim on in1 keeps
                    # the op eligible for the DVE 2x perf mode.
                    dlp = msgs_pool.tile([128, 2 * max_ptiles], bf16, tag="dlp")
                    nc.sync.dma_start(
                        out=dlp[:, : 2 * ptiles],
                        in_=dl_in[:, 2 * first_g_p : 2 * (first_g_p + ptiles)],
                    )
                    s_built = msgs_pool.tile(
                        [128, max_ptiles * 256], bf16, tag="sb", bufs=1
                    )
                    for t0 in range(0, ptiles, SBATCH):
                        tb = min(SBATCH, ptiles - t0)
                        nc.vector.tensor_tensor(
                            out=s_built[:, t0 * 256 : (t0 + tb) * 256].rearrange(
                                "p (t jh jl) -> p t jh jl", jl=2, jh=128
                            ),
                            in0=iota_sb[:, : tb * 256].rearrange(
                                "p (t jh jl) -> p t jh jl", jl=2, jh=128
                            ),
                            in1=dlp[:, 2 * t0 : 2 * (t0 + tb)].rearrange(
                                "p (t o jl) -> p t o jl", jl=2, o=1
                            ).to_broadcast([128, tb, 128, 2]),
                            op=Alu.is_equal,
                        )
                    msgs16 = msgs_pool.tile(
                        [128, max_ptiles, 2 * H], bf16, tag="msgs16"
                    )
                    GCAP = 8192  # max idxs per dma_gather (desc-ring capacity)
                    for kk in range(NCHUNK):
                        off_sl, n_sl, col = call_info[p][kk]
                        for o in range(0, n_sl, GCAP):
                            ln = min(GCAP, n_sl - o)
                            c0 = col + o // 128
                            i0 = (off_sl + o) // 16 - p_off16
                            nc.gpsimd.dma_gather(
                                msgs16[:, c0 : c0 + ln // 128, :],
                                tables[hop][
                                    CH_PAD * kk : CH_PAD * (kk + 1), :
                                ].rearrange("(n s) h -> n (s h)", s=2),
                                idxp[:, i0 : i0 + ln // 16],
                                ln,
                                ln,
                                2 * H,
                                single_packet=False,
                                queue_num=kk,
                            )
                    first_g = g
                    # matmuls per window
                    for ww in range(p * G_WIN, (p + 1) * G_WIN):
                        agg_ps = psum_pool.tile([128, 128], f32, tag="ps", bufs=8)
                        n_t = int(sum(T[ww][kk] for kk in range(NCHUNK)))
                        n_mm = 2 * n_t
                        mm_i = 0
                        for kk in range(NCHUNK):
                            _, _, col = call_info[p][kk]
                            cbase = col + int(
                                sum(T[w2][kk] for w2 in range(p * G_WIN, ww))
                            )
                            for t in range(int(T[ww][kk])):
                                # global tile index in host (p,k,w,t) order
                                lg = int(tile_start[ww][kk]) + t - first_g
                                for half in range(2):
                                    nc.tensor.matmul(
                                        out=agg_ps[:, :H],
                                        lhsT=s_built[
                                            :,
                                            lg * 256 + half * 128
                                            : lg * 256 + (half + 1) * 128,
                                        ],
                                        rhs=msgs16[
                                            :, cbase + t,
                                            half * H : (half + 1) * H,
                                        ],
                                        start=(mm_i == 0),
                                        stop=(mm_i == n_mm - 1),
                                    )
                                    mm_i += 1
                        g += n_t
                        # u' = u - dinv2 * agg  (scale on ACT, subtract on DVE)
                        tscl = work.tile([128, H], bf16, tag="tscl", bufs=4)
                        nc.scalar.activation(
                            out=tscl[:], in_=agg_ps[:, :H],
                            func=mybir.ActivationFunctionType.Copy,
                            scale=dinv2_sb[:, ww : ww + 1],
                        )
                        nc.vector.tensor_tensor(
                            out=nxt[:, ww * H : (ww + 1) * H],
                            in0=cur[:, ww * H : (ww + 1) * H],
                            in1=tscl[:],
                            op=Alu.subtract,
                        )
                    if hop < NHOP - 1:
                        # stage this phase's (bf16) u slice into ag_h right
                        # away, so the hop boundary pays nothing before the
                        # AllGather launches
                        cs2 = slice(p * G_WIN * H, (p + 1) * G_WIN * H)
                        w0p = p * G_WIN
                        nc.sync.dma_start(
                            out=ag_h[
                                w0p * 128 : (w0p + G_WIN) * 128, :
                            ].rearrange("(w p) h -> p w h", p=128),
                            in_=nxt[:, cs2].rearrange(
                                "p (w h) -> p w h", h=H
                            ),
                        )
                    if hop == NHOP - 1:
                        # combine this phase's windows now (u3 slice just
                        # written; u0/u1 from DRAM, u2 = cur) -- hides the
                        # epilogue under the gather-bound last hop
                        w0 = p * G_WIN
                        cs = slice(w0 * H, (w0 + G_WIN) * H)
                        u0c = work.tile([128, G_WIN * H], bf16, tag="u0c",
                                        bufs=2)
                        nc.sync.dma_start(out=u0c[:], in_=saves[0][:, cs])
                        u1c = work.tile([128, G_WIN * H], bf16, tag="u1c",
                                        bufs=2)
                        nc.sync.dma_start(out=u1c[:], in_=saves[1][:, cs])
                        us = [u0c[:], u1c[:], cur[:, cs], nxt[:, cs]]
                        out_st = work.tile([128, G_WIN, 4 * H], f32,
                                           tag="outst", bufs=2)
                        for i in range(4):
                            acc = work.tile([128, G_WIN * H], f32, tag="acc",
                                            bufs=2)
                            a = acc[:]
                            ks = [kk for kk in range(4)
                                  if THETAS[i][kk] != 0.0]
                            nc.scalar.activation(
                                out=a, in_=us[ks[0]],
                                func=mybir.ActivationFunctionType.Copy,
                                scale=float(THETAS[i][ks[0]]),
                            )
                            for kk in ks[1:]:
                                tmp = work.tile([128, G_WIN * H], f32,
                                                tag="ctmp", bufs=2)
                                nc.scalar.activation(
                                    out=tmp[:], in_=us[kk],
                                    func=mybir.ActivationFunctionType.Copy,
                                    scale=float(THETAS[i][kk]),
                                )
                                nc.vector.tensor_tensor(
                                    out=a, in0=a, in1=tmp[:], op=Alu.add
                                )
                            nc.vector.tensor_tensor(
                                out=out_st[:, :, i * H : (i + 1) * H],
                                in0=a.rearrange("p (w h) -> p w h", h=H),
                                in1=dsqrt_sb[:, w0 : w0 + G_WIN].to_broadcast(
                                    [128, G_WIN, H]
                                ),
                                op=Alu.mult,
                            )
                        nc.sync.dma_start(
                            out=out[
                                w0 * 128 : (w0 + G_WIN) * 128, :
                            ].rearrange("(w p) h -> p w h", p=128),
                            in_=out_st[:],
                        )
                assert g == sum(phase_tiles[:NPHASE])
                if hop < NHOP - 1:
                    launch_ag(hop + 1)
                if hop == 0:
                    nc.sync.dma_start(out=saves[1][:], in_=nxt[:])  # save u1
                cur, nxt = nxt, cur


# ---------------------------------------------------------------- runner
def _make_runner(nc, in_maps, n_cores):
    import jax
    import numpy as np
    from jax.sharding import Mesh, NamedSharding, PartitionSpec
    from jax.experimental.shard_map import shard_map

    import concourse.mybir as mybir
    from concourse import bass2jax

    bass2jax.install_neuronx_cc_hook()
    partition_name = nc.partition_id_tensor.name if nc.partition_id_tensor else None
    in_names, out_names, out_avals, zero_outs = [], [], [], []
    for alloc in nc.m.functions[0].allocations:
        if not isinstance(alloc, mybir.MemoryLocationSet):
            continue
        name = alloc.memorylocations[0].name
        if alloc.kind == "ExternalInput":
            if name != partition_name:
                in_names.append(name)
        elif alloc.kind == "ExternalOutput":
            out_names.append(name)
            shape = tuple(alloc.tensor_shape)
            dtype = mybir.dt.np(alloc.dtype)
            out_avals.append(jax.core.ShapedArray(shape, dtype))
            zero_outs.append(np.zeros(shape, dtype))
    n_params = len(in_names)
    all_in_names = list(in_names) + list(out_names)
    if partition_name is not None:
        all_in_names.append(partition_name)

    def _body(*args):
        operands = list(args)
        if partition_name is not None:
            operands.append(bass2jax.partition_id_tensor())
        outs = bass2jax._bass_exec_p.bind(
            *operands,
            out_avals=tuple(out_avals),
            in_names=tuple(all_in_names),
            out_names=tuple(out_names),
            lowering_input_output_aliases=(),
            sim_require_finite=True,
            sim_require_nnan=True,
            nc=nc,
        )
        return tuple(outs)

    devices = jax.devices()[:n_cores]
    mesh = Mesh(np.asarray(devices), ("core",))
    n_ops = n_params + len(out_names)
    sharded = jax.jit(
        shard_map(
            _body,
            mesh=mesh,
            in_specs=(PartitionSpec("core"),) * n_ops,
            out_specs=(PartitionSpec("core"),) * len(out_names),
            check_rep=False,
        ),
        keep_unused=True,
    )
    sh = NamedSharding(mesh, PartitionSpec("core"))
    concat_in = [
        jax.device_put(
            np.concatenate([np.asarray(in_maps[c][nm]) for c in range(n_cores)], 0),
            sh,
        )
        for nm in in_names
    ]
    concat_zeros = [
        jax.device_put(np.zeros((n_cores * z.shape[0], *z.shape[1:]), z.dtype), sh)
        for z in zero_outs
    ]
    args = concat_in + concat_zeros

    def run():
        return sharded(*args)

    return run, out_names, out_avals


_CACHE = {}


def _get_built(edge_index_bytes_key, edge_index):
    if edge_index_bytes_key not in _CACHE:
        prep = _prep(edge_index)
        nc = _build_nc(
            prep["T"],
            prep["tot_tiles"],
            prep["phase_tiles"],
            prep["call_info"],
            prep["tile_start"],
        )
        _CACHE[edge_index_bytes_key] = (prep, nc)
    return _CACHE[edge_index_bytes_key]


def _make_in_maps(prep, inputs):
    import ml_dtypes

    feature = np.asarray(inputs["feature"], dtype=np.float32)
    W = np.asarray(inputs["W"], dtype=np.float32)
    b = np.asarray(inputs["b"], dtype=np.float32)

    b2 = b.reshape(1, H)
    iota = np.tile(np.arange(256, dtype=np.float32), SBATCH).reshape(1, -1)
    iota = np.repeat(iota, 128, axis=0).astype(ml_dtypes.bfloat16)

    pos = prep["pos"]
    feat_pad = np.zeros((NCORES, NPC_PAD, F_IN), dtype=np.float32)
    fv = feature.reshape(NCORES, NPC, F_IN)
    for c in range(NCORES):
        feat_pad[c, pos[c], :] = fv[c]
    # transposed per-core features [F, NPC_PAD] f32
    featT = np.ascontiguousarray(feat_pad.transpose(0, 2, 1))
    # transposed full features (global core-major order), bf16, same for all
    featfullT = np.ascontiguousarray(
        feat_pad.reshape(NCORES * NPC_PAD, F_IN).T.astype(ml_dtypes.bfloat16)
    )

    in_maps = []
    for c in range(NCORES):
        idx_w, dl_t = prep["per_core"][c]
        in_maps.append(
            {
                "featT_in": featT[c],
                "featfullT_in": featfullT,
                "w_in": W,
                "b_in": b2,
                "iota_in": iota,
                "idx_in": idx_w,
                "dl_in": dl_t,
                "dinv_in": prep["dinv_t"][c],
                "dinv2_in": prep["dinv2_t"][c],
                "dsqrt_in": prep["dsqrt_t"][c],
                "dinvfull_in": prep["dinv_full"],
            }
        )
    return in_maps


def kernel(feature, edge_index, W, b):
    import jax

    edge_index = np.asarray(edge_index, dtype=np.int32)
    key = hash(edge_index.tobytes())
    prep, nc = _get_built(key, edge_index)
    in_maps = _make_in_maps(prep, {"feature": feature, "W": W, "b": b})

    run, out_names, out_avals = _make_runner(nc, in_maps, NCORES)
    outs = jax.block_until_ready(run())
    oi = out_names.index("out")
    full = np.asarray(outs[oi]).reshape(NCORES, NPC_PAD, 4 * H)
    pos = prep["pos"]
    res = np.empty((NCORES, NPC, 4 * H), np.float32)
    for c in range(NCORES):
        res[c] = full[c, pos[c], :]
    return res.reshape(N, 4 * H)


if __name__ == "__main__":
    rng = np.random.default_rng(0)
    feature = rng.standard_normal((N, F_IN), dtype=np.float32)
    edge_index = rng.integers(0, N, (2, E)).astype(np.int32)
    W = (rng.standard_normal((F_IN, H)) * 0.05).astype(np.float32)
    b = (rng.standard_normal((H,)) * 0.05).astype(np.float32)
    out = kernel(feature=feature, edge_index=edge_index, W=W, b=b)
    print(out.shape, out.dtype, float(np.abs(out).mean()))
